# revision 16
# baseline (speedup 1.0000x reference)
"""Trainium2 Bass kernel v5 for nn_AngularDescriptor (gnn_message_passing).

Legendre-addition-theorem factorization: q[i,d,l] = sum_{m in shell l}
A[i,d,m]^2 - B[i,d] with A = sum_j g_ij Y_m(u_ij), B = sum_j g_ij^2,
g scaled by sqrt(0.5) via the host-marshaled c rows.

All index gathers (neighbor positions, c_table rows) happen on the
host; the device kernel is a pure DVE pipeline working full-width over
all S*M = 200 pairs per partition.  v5 notes (from v3/v4 traces):
 - GPSIMD shares its SBUF port with the DVE, so concurrent GPSIMD
   elementwise work halves DVE throughput: everything runs on V.
 - the 3.3MB cj DMA is gated behind a dummy 1-element copy that
   depends on posj, so the small geometry inputs get the full fabric
   and V starts ~10us earlier.
 - Chebyshev recurrence, Y scratch, and g^2 run in bf16 (2x DVE modes);
   numpy emulation puts the accuracy cost at ~1e-3.
 - strided bf16 *writes* are ~4x slow; every op writes contiguously.
"""
import os
import sys

sys.path.insert(0, "/opt/trn_rl_repo")
os.environ.setdefault("NEURON_RT_RESET_CORES", "1")

import math
import numpy as np

from concourse import bacc, bass, mybir, tile
from concourse.bass_utils import run_bass_kernel_spmd

# problem constants
N_ATOMS = 10000
M_NBR = 20
N_TYPES = 4
N_DESC = 8
K_MAX = 8
L_MAX = 4
R_C = 5.0

NCORES = 8
P = 128
S = 10                      # atom slots per partition
CA = P * S                  # atoms per core = 1280
NTOT = NCORES * CA          # padded atom count = 10240
PAIRS = S * M_NBR           # 200 pairs per partition

F32 = mybir.dt.float32
BF16 = mybir.dt.bfloat16

SQ3 = math.sqrt(3.0)
C31 = math.sqrt(3.0 / 8.0)
C32 = math.sqrt(15.0)
C33 = math.sqrt(5.0 / 8.0)
SHELL_OFF = [0, 1, 4, 9, 16]

AF = mybir.ActivationFunctionType
ALU = mybir.AluOpType
AX = mybir.AxisListType


def _ap(t, off, dims):
    base = t[:]
    ap = [list(base.ap[0])] + [[s, c] for (s, c) in dims]
    return bass.AP(base.tensor, base.offset + off, ap)


def build_nc(debug=False):
    nc = bacc.Bacc()
    posj_d = nc.declare_dram_parameter("posj", [P, PAIRS * 3], F32,
                                       isOutput=False)
    ctr_d = nc.declare_dram_parameter("ctr", [P, S * 3], F32, isOutput=False)
    cj_d = nc.declare_dram_parameter("cj", [P, PAIRS * 64], BF16,
                                     isOutput=False)
    out_d = nc.declare_dram_parameter("out", [P, S * N_DESC * L_MAX], F32,
                                      isOutput=True)

    with tile.TileContext(nc) as tc:
        with tc.tile_pool(name="main", bufs=1) as pool:
            # activation biases must be const APs
            cvals = [math.pi / 2, 0.0]
            consts = pool.tile([P, len(cvals)], F32)
            for ci, cv in enumerate(cvals):
                nc.vector.memset(consts[:, ci:ci + 1], cv)
                nc.const_aps.aps[(F32, cv)] = consts[:, ci:ci + 1]

            ph1 = tc.tile_pool(name="ph1", bufs=1)
            cp = ph1.__enter__()

            ctr = pool.tile([P, S * 3], F32)
            posj = cp.tile([P, PAIRS * 3], F32)    # [s, 3, j]
            nc.sync.dma_start(out=posj[:], in_=posj_d[:])
            nc.sync.dma_start(out=ctr[:], in_=ctr_d[:])
            cj = cp.tile([P, PAIRS * 64], BF16)    # [s, j, d, k]
            # gate the big cj DMA behind posj via a WAW dependency: the
            # dummy copy reads posj, writes one element of cj, and the
            # DMA (which overwrites all of cj) must follow it.
            with nc.allow_low_precision(reason="dma gate dummy"):
                nc.vector.tensor_copy(out=cj[:, 0:1], in_=posj[:, 0:1])
            nc.scalar.dma_start(out=cj[:], in_=cj_d[:])

            # early memsets while waiting for posj
            Y = pool.tile([P, PAIRS * 16], BF16)   # [s, m, j]

            def y_slice(m, cnt=1):
                if cnt == 1:
                    return _ap(Y, m * M_NBR, [(16 * M_NBR, S), (1, M_NBR)])
                return _ap(Y, m * M_NBR,
                           [(16 * M_NBR, S), (M_NBR, cnt), (1, M_NBR)])

            Tall = cp.tile([P, K_MAX * PAIRS], BF16)   # [k][s,j]

            def t_slice(k):
                return _ap(Tall, k * PAIRS, [(1, PAIRS)])

            with nc.allow_low_precision(reason="bf16 basis"):
                nc.vector.memset(y_slice(0), 1.0)
                nc.vector.memset(t_slice(0), 1.0)

            # ---- geometry (f32): dxyz, r2, r, 1/r -----------------------
            C3 = PAIRS * 3
            SJ3 = 3 * M_NBR

            def c_slice(t, c):
                return _ap(t, c * M_NBR, [(SJ3, S), (1, M_NBR)])

            dxyz = cp.tile([P, C3], BF16)          # [s, 3, j]
            sq = cp.tile([P, C3], BF16)
            r2 = cp.tile([P, PAIRS], BF16)
            with nc.allow_low_precision(reason="bf16 geometry"):
                nc.vector.tensor_tensor(
                    out=dxyz[:], in0=posj[:],
                    in1=_ap(ctr, 0, [(3, S), (1, 3), (0, M_NBR)]),
                    op=ALU.subtract)
                nc.vector.tensor_tensor(out=sq[:], in0=dxyz[:], in1=dxyz[:],
                                        op=ALU.mult)
                nc.vector.tensor_tensor(out=r2[:], in0=c_slice(sq, 0),
                                        in1=c_slice(sq, 1), op=ALU.add)
                nc.vector.tensor_tensor(
                    out=r2[:],
                    in0=_ap(r2, 0, [(M_NBR, S), (1, M_NBR)]),
                    in1=c_slice(sq, 2), op=ALU.add)
            # clamp r^2 at R_C^2: fc(R_C) = 0 exactly, so pairs beyond
            # the cutoff contribute 0 through fch regardless of u/tm1.
            with nc.allow_low_precision(reason="bf16 geometry"):
                nc.vector.tensor_scalar_min(out=r2[:], in0=r2[:],
                                            scalar1=R_C * R_C)
            rr = cp.tile([P, PAIRS], F32)
            nc.scalar.sqrt(out=rr[:], in_=r2[:])
            cosx = cp.tile([P, PAIRS], F32)
            nc.scalar.activation(out=cosx[:], in_=rr[:], func=AF.Sin,
                                 bias=math.pi / 2, scale=-math.pi / R_C)
            rinv = cp.tile([P, PAIRS], F32)
            nc.vector.reciprocal(out=rinv[:], in_=rr[:])
            u = cp.tile([P, C3], BF16)             # [s, 3, j] unit vectors
            with nc.allow_low_precision(reason="bf16 unit vectors"):
                nc.vector.tensor_tensor(
                    out=u[:], in0=dxyz[:],
                    in1=_ap(rinv, 0, [(M_NBR, S), (0, 3), (1, M_NBR)]),
                    op=ALU.mult)

            # ---- Chebyshev recurrence in bf16 [k][s,j] ------------------
            tm1 = cp.tile([P, PAIRS], F32)
            nc.vector.tensor_scalar(out=tm1[:], in0=rr[:], scalar1=1.0 / R_C,
                                    scalar2=-1.0, op0=ALU.mult, op1=ALU.add)
            x2 = cp.tile([P, PAIRS], BF16)
            with nc.allow_low_precision(reason="bf16 chebyshev"):
                # T1 = 2*tm1^2 - 1 written into Tall[1]
                nc.vector.scalar_tensor_tensor(out=t_slice(1), in0=tm1[:],
                                               scalar=2.0, in1=tm1[:],
                                               op0=ALU.mult, op1=ALU.mult)
                nc.vector.tensor_scalar(out=t_slice(1), in0=t_slice(1),
                                        scalar1=-1.0, scalar2=None,
                                        op0=ALU.add)
                nc.vector.tensor_scalar_mul(out=x2[:], in0=t_slice(1),
                                            scalar1=2.0)
                for k in range(2, K_MAX):
                    nc.vector.tensor_tensor(out=t_slice(k), in0=x2[:],
                                            in1=t_slice(k - 1), op=ALU.mult)
                    if k == 2:
                        nc.vector.tensor_scalar(out=t_slice(k),
                                                in0=t_slice(k), scalar1=-1.0,
                                                scalar2=None, op0=ALU.add)
                    else:
                        nc.vector.tensor_tensor(out=t_slice(k),
                                                in0=t_slice(k),
                                                in1=t_slice(k - 2),
                                                op=ALU.subtract)

            # fch = 0.25*cosx + 0.25  (= 0.5 * fc), bf16
            fch = cp.tile([P, PAIRS], BF16)
            with nc.allow_low_precision(reason="bf16 fch"):
                nc.vector.tensor_scalar(out=fch[:], in0=cosx[:], scalar1=0.25,
                                        scalar2=0.25, op0=ALU.mult,
                                        op1=ALU.add)

            # ---- radial basis f[k][s,j] = Tall[k] * fch -----------------
            # k-major like Tall, so this op is fully contiguous (2x mode).
            # (k=0 slice of Tall is 1, so f[0] = fch; the "+1" of the
            # reference basis is folded into c column 0 on the host)
            f = pool.tile([P, K_MAX * PAIRS], BF16)
            with nc.allow_low_precision(reason="bf16 radial basis"):
                nc.vector.tensor_tensor(
                    out=f[:], in0=Tall[:],
                    in1=_ap(fch, 0, [(0, K_MAX), (1, PAIRS)]),
                    op=ALU.mult)

            # ---- spherical harmonics Y[s,m,j] (bf16) --------------------
            # within-shell m order: l=2 -> [xy, xz, yz, z2, x2-y2]
            #                       l=3 -> [z3, x(5z2-1), y(5z2-1),
            #                               (x2-y2)z, xyz, tl5*x, tl6*y]
            def u_c(c, cnt=1):
                if cnt == 1:
                    return c_slice(u, c)
                return _ap(u, c * M_NBR, [(SJ3, S), (M_NBR, cnt), (1, M_NBR)])

            sc = cp.tile([P, PAIRS * 6], BF16)
            zs = cp.tile([P, PAIRS], BF16)

            def sc_t(i, cnt=1):
                if cnt == 1:
                    return _ap(sc, i * PAIRS, [(1, PAIRS)])
                return _ap(sc, i * PAIRS, [(M_NBR, S), (PAIRS, cnt),
                                           (1, M_NBR)])

            x2c, y2c, z2c, xyc, dxyc, tl = [sc_t(i) for i in range(6)]
            with nc.allow_low_precision(reason="bf16 Y"):
                nc.vector.tensor_copy(
                    out=y_slice(1, 3),
                    in_=_ap(u, 0, [(SJ3, S), (M_NBR, 3), (1, M_NBR)]))
                # x2c, y2c, z2c in one op
                nc.vector.tensor_tensor(out=sc_t(0, 3), in0=u_c(0, 3),
                                        in1=u_c(0, 3), op=ALU.mult)
                nc.vector.tensor_tensor(out=xyc, in0=u_c(0), in1=u_c(1),
                                        op=ALU.mult)
                nc.vector.tensor_scalar_mul(out=y_slice(4), in0=xyc,
                                            scalar1=SQ3)
                # Y5 = sqrt3*uz*ux, Y6 = sqrt3*uz*uy via pre-scaled uz
                # (STT is limited to 2 free dims; TT allows the pair)
                nc.vector.tensor_scalar_mul(out=zs[:], in0=u_c(2),
                                            scalar1=SQ3)
                nc.vector.tensor_tensor(
                    out=y_slice(5, 2),
                    in0=_ap(zs, 0, [(M_NBR, S), (0, 2), (1, M_NBR)]),
                    in1=u_c(0, 2), op=ALU.mult)
                nc.vector.tensor_scalar(out=y_slice(7), in0=z2c, scalar1=1.5,
                                        scalar2=-0.5, op0=ALU.mult,
                                        op1=ALU.add)
                nc.vector.tensor_tensor(out=dxyc, in0=x2c, in1=y2c,
                                        op=ALU.subtract)
                nc.vector.tensor_scalar_mul(out=y_slice(8), in0=dxyc,
                                            scalar1=SQ3 / 2)
                nc.vector.tensor_scalar(out=tl, in0=z2c, scalar1=2.5,
                                        scalar2=-1.5, op0=ALU.mult,
                                        op1=ALU.add)
                nc.vector.tensor_tensor(out=y_slice(9), in0=tl, in1=u_c(2),
                                        op=ALU.mult)
                nc.vector.tensor_scalar(out=tl, in0=z2c, scalar1=5.0 * C31,
                                        scalar2=-C31, op0=ALU.mult,
                                        op1=ALU.add)
                # Y10 = tl*ux, Y11 = tl*uy in one op
                nc.vector.tensor_tensor(
                    out=y_slice(10, 2),
                    in0=_ap(sc, 5 * PAIRS, [(M_NBR, S), (0, 2), (1, M_NBR)]),
                    in1=u_c(0, 2), op=ALU.mult)
                nc.vector.scalar_tensor_tensor(out=y_slice(12), in0=dxyc,
                                               scalar=C32 / 2, in1=u_c(2),
                                               op0=ALU.mult, op1=ALU.mult)
                nc.vector.scalar_tensor_tensor(out=y_slice(13), in0=xyc,
                                               scalar=C32, in1=u_c(2),
                                               op0=ALU.mult, op1=ALU.mult)
                nc.vector.scalar_tensor_tensor(out=tl, in0=y2c, scalar=3.0,
                                               in1=x2c, op0=ALU.mult,
                                               op1=ALU.subtract)
                nc.vector.scalar_tensor_tensor(out=y_slice(14), in0=tl,
                                               scalar=-C33, in1=u_c(0),
                                               op0=ALU.mult, op1=ALU.mult)
                nc.vector.scalar_tensor_tensor(out=tl, in0=x2c, scalar=3.0,
                                               in1=y2c, op0=ALU.mult,
                                               op1=ALU.subtract)
                nc.vector.scalar_tensor_tensor(out=y_slice(15), in0=tl,
                                               scalar=C33, in1=u_c(1),
                                               op0=ALU.mult, op1=ALU.mult)

            # ---- g[s,d,j] = sum_k cj[k,s,d,j] * f[k,s,j] ----------------
            # cj is host-marshaled k-major so the whole k tree is flat
            # contiguous adds (2x mode) and g lands in [s,d,j] directly.
            x2t = cp.tile([P, PAIRS * 64], BF16)     # [k, s, d, j]
            a4 = cp.tile([P, PAIRS * 32], BF16)      # [k4, s, d, j]
            a2 = cp.tile([P, PAIRS * 16], BF16)      # [k2, s, d, j]
            g = pool.tile([P, PAIRS * N_DESC], BF16)   # [s, d, j]
            with nc.allow_low_precision(reason="bf16 contraction"):
                nc.vector.tensor_tensor(
                    out=_ap(x2t, 0, [(1600, K_MAX), (160, S), (20, N_DESC),
                                     (1, M_NBR)]),
                    in0=_ap(cj, 0, [(1600, K_MAX), (160, S), (20, N_DESC),
                                    (1, M_NBR)]),
                    in1=_ap(f, 0, [(PAIRS, K_MAX), (M_NBR, S), (0, N_DESC),
                                   (1, M_NBR)]),
                    op=ALU.mult)
                nc.vector.tensor_tensor(
                    out=a4[:], in0=x2t[:, 0:6400], in1=x2t[:, 6400:12800],
                    op=ALU.add)
                nc.vector.tensor_tensor(
                    out=a2[:], in0=a4[:, 0:3200], in1=a4[:, 3200:6400],
                    op=ALU.add)
                nc.vector.tensor_tensor(
                    out=g[:], in0=a2[:, 0:1600], in1=a2[:, 1600:3200],
                    op=ALU.add)

            ph1.__exit__(None, None, None)
            ph2 = tc.tile_pool(name="ph2", bufs=1)
            p2 = ph2.__enter__()

            # ---- A[s,d,m] = sum_j g * Y  (tree over j = 8+8+4) ----------
            # m has 17 slots: 16 spherical harmonics plus g^2 (so the
            # tree also produces B[s,d] = A[s,d,16] for free).
            MM = 17
            xa = p2.tile([P, S * N_DESC * MM * M_NBR], BF16)  # [s,d,m,j]
            t8 = p2.tile([P, S * N_DESC * MM * 8], BF16)
            t4 = p2.tile([P, S * N_DESC * MM * 4], BF16)
            t4b = p2.tile([P, S * N_DESC * MM * 4], BF16)
            t2 = p2.tile([P, S * N_DESC * MM * 2], BF16)
            A = pool.tile([P, S * N_DESC * MM], F32)          # [s, d, m17]
            SX, DX = N_DESC * MM * M_NBR, MM * M_NBR
            with nc.allow_low_precision(reason="bf16 outer product"):
                # g^2 into m=16 (contiguous both sides -> 2x mode)
                nc.vector.tensor_tensor(
                    out=_ap(xa, 16 * M_NBR, [(SX, S), (DX, N_DESC),
                                             (1, M_NBR)]),
                    in0=_ap(g, 0, [(160, S), (20, N_DESC), (1, M_NBR)]),
                    in1=_ap(g, 0, [(160, S), (20, N_DESC), (1, M_NBR)]),
                    op=ALU.mult)
                nc.vector.tensor_tensor(
                    out=_ap(xa, 0, [(SX, S), (DX, N_DESC), (20, 16),
                                    (1, 20)]),
                    in0=_ap(g, 0, [(160, S), (20, N_DESC), (0, 16), (1, 20)]),
                    in1=_ap(Y, 0, [(320, S), (0, N_DESC), (20, 16), (1, 20)]),
                    op=ALU.mult)
                nc.vector.tensor_tensor(
                    out=_ap(t8, 0, [(MM * 64, S), (MM * 8, N_DESC), (8, MM),
                                    (1, 8)]),
                    in0=_ap(xa, 0, [(SX, S), (DX, N_DESC), (20, MM),
                                    (1, 8)]),
                    in1=_ap(xa, 8, [(SX, S), (DX, N_DESC), (20, MM),
                                    (1, 8)]),
                    op=ALU.add)
                nc.vector.tensor_tensor(
                    out=_ap(t4, 0, [(MM * 32, S), (MM * 4, N_DESC), (4, MM),
                                    (1, 4)]),
                    in0=_ap(t8, 0, [(MM * 64, S), (MM * 8, N_DESC), (8, MM),
                                    (1, 4)]),
                    in1=_ap(t8, 4, [(MM * 64, S), (MM * 8, N_DESC), (8, MM),
                                    (1, 4)]),
                    op=ALU.add)
                nc.vector.tensor_tensor(
                    out=_ap(t4b, 0, [(MM * 32, S), (MM * 4, N_DESC), (4, MM),
                                     (1, 4)]),
                    in0=_ap(t4, 0, [(MM * 32, S), (MM * 4, N_DESC), (4, MM),
                                    (1, 4)]),
                    in1=_ap(xa, 16, [(SX, S), (DX, N_DESC), (20, MM),
                                     (1, 4)]),
                    op=ALU.add)
                nc.vector.tensor_tensor(
                    out=_ap(t2, 0, [(MM * 16, S), (MM * 2, N_DESC), (2, MM),
                                    (1, 2)]),
                    in0=_ap(t4b, 0, [(MM * 32, S), (MM * 4, N_DESC), (4, MM),
                                     (1, 2)]),
                    in1=_ap(t4b, 2, [(MM * 32, S), (MM * 4, N_DESC), (4, MM),
                                     (1, 2)]),
                    op=ALU.add)
            nc.vector.tensor_tensor(
                out=_ap(A, 0, [(N_DESC * MM, S), (MM, N_DESC), (1, MM)]),
                in0=_ap(t2, 0, [(MM * 16, S), (MM * 2, N_DESC), (2, MM)]),
                in1=_ap(t2, 1, [(MM * 16, S), (MM * 2, N_DESC), (2, MM)]),
                op=ALU.add)

            if debug:
                for nm, t, dt in [("f", f, BF16), ("Y", Y, BF16),
                                  ("g", g, BF16), ("A", A, F32)]:
                    dd = nc.declare_dram_parameter(
                        "d_" + nm, [P, t.shape[1]], dt, isOutput=True)
                    nc.sync.dma_start(out=dd[:], in_=t[:])

            # ---- q[s,d,l] = sum_{m in shell l} A^2 - B ------------------
            # two atom-halves so the first half's output DMA overlaps the
            # second half's compute
            Asq = pool.tile([P, S * N_DESC * 16], F32)
            outq = pool.tile([P, S * N_DESC * L_MAX], F32)
            q2la = pool.tile([P, S * N_DESC * L_MAX], F32)
            SH_ = S // 2
            for h in range(2):
                sqo, ao, qo = h * SH_ * 128, h * SH_ * N_DESC * MM,                     h * SH_ * N_DESC * L_MAX
                nc.vector.tensor_tensor(
                    out=_ap(Asq, sqo, [(128, SH_), (16, N_DESC), (1, 16)]),
                    in0=_ap(A, ao, [(N_DESC * MM, SH_), (MM, N_DESC),
                                    (1, 16)]),
                    in1=_ap(A, ao, [(N_DESC * MM, SH_), (MM, N_DESC),
                                    (1, 16)]),
                    op=ALU.mult)
                for l in range(L_MAX):
                    cnt = SHELL_OFF[l + 1] - SHELL_OFF[l]
                    nc.vector.tensor_reduce(
                        out=_ap(q2la, qo + l, [(N_DESC * L_MAX, SH_),
                                               (L_MAX, N_DESC)]),
                        in_=_ap(Asq, sqo + SHELL_OFF[l],
                                [(N_DESC * 16, SH_), (16, N_DESC), (1, cnt)]),
                        axis=AX.X, op=ALU.add)
                nc.vector.tensor_tensor(
                    out=_ap(outq, qo, [(1, SH_ * N_DESC * L_MAX)]),
                    in0=_ap(q2la, qo, [(1, SH_ * N_DESC * L_MAX)]),
                    in1=_ap(A, ao + 16, [(N_DESC * MM, SH_), (MM, N_DESC),
                                         (0, L_MAX)]),
                    op=ALU.subtract)
                nq = SH_ * N_DESC * L_MAX
                nc.sync.dma_start(out=out_d[:, qo:qo + nq],
                                  in_=outq[:, qo:qo + nq])
            ph2.__exit__(None, None, None)
    nc.finalize()
    return nc


def make_inputs(types, positions, angular_neighbors, c_table):
    types = np.asarray(types).astype(np.int64)
    positions = np.ascontiguousarray(np.asarray(positions, dtype=np.float32))
    nbr = np.asarray(angular_neighbors).astype(np.int64)
    c_table = np.asarray(c_table, dtype=np.float32)
    import ml_dtypes

    pad = NTOT - N_ATOMS
    types_pad = np.concatenate([types, np.repeat(types[-1:], pad, 0)], 0)
    pos_pad = np.concatenate([positions, np.repeat(positions[-1:], pad, 0)],
                             0)
    nbr_pad = np.concatenate([nbr, np.repeat(nbr[-1:], pad, 0)], 0)

    # per-(t_i,t_j) c rows in [d, k] order, sqrt(0.5)-scaled.
    # Column 0 absorbs sum_k c[d,k] (device basis is [fch, T_1*fch, ...]).
    c_adj = c_table.astype(np.float64).copy()
    c_adj[..., 0] += c_table.astype(np.float64).sum(-1)
    c16 = (c_adj * math.sqrt(0.5)).astype(ml_dtypes.bfloat16)  # [4,4,8,8]

    pvec = np.arange(P)
    svec = np.arange(S)
    in_maps = []
    for c in range(NCORES):
        atom = c * CA + pvec[:, None] * S + svec[None, :]       # [P, S]
        nbrs = nbr_pad[atom]                                    # [P, S, 20]
        ctr = pos_pad[atom].reshape(P, S * 3).astype(np.float32)
        # component-major: [P, S, 3, M]
        posj = pos_pad[nbrs].transpose(0, 1, 3, 2).reshape(P, PAIRS * 3)
        tj = types_pad[nbrs]                                    # [P, S, 20]
        # [P,S,M,d,k] -> k-major [P,k,S,d,M(j)]
        cjf = c16[types_pad[atom][:, :, None], tj]
        cj = cjf.transpose(0, 4, 1, 3, 2).reshape(P, PAIRS * 64)
        in_maps.append({
            "posj": np.ascontiguousarray(posj.astype(np.float32)),
            "ctr": np.ascontiguousarray(ctr),
            "cj": np.ascontiguousarray(cj),
        })
    return in_maps


_NC_CACHE = None


def kernel(types, positions, angular_neighbors, c_table):
    global _NC_CACHE
    in_maps = make_inputs(types, positions, angular_neighbors, c_table)
    if _NC_CACHE is None:
        _NC_CACHE = build_nc()
    res = run_bass_kernel_spmd(_NC_CACHE, in_maps,
                               core_ids=list(range(NCORES)))
    outs = [res.results[c]["out"].reshape(CA, N_DESC, L_MAX)
            for c in range(NCORES)]
    q = np.concatenate(outs, 0)[:N_ATOMS]
    return np.ascontiguousarray(q.astype(np.float32))


if __name__ == "__main__":
    z = np.load("/tmp/ref_cache.npz")
    inputs = {k: z[k] for k in
              ("types", "positions", "angular_neighbors", "c_table")}
    exp = z["exp"]
    act = kernel(**inputs)
    rel = np.linalg.norm(act - exp) / np.linalg.norm(exp)
    print("Relative error:", rel)


# revision 17
# speedup vs baseline: 1.0098x; 1.0098x over previous
"""Trainium2 Bass kernel v5 for nn_AngularDescriptor (gnn_message_passing).

Legendre-addition-theorem factorization: q[i,d,l] = sum_{m in shell l}
A[i,d,m]^2 - B[i,d] with A = sum_j g_ij Y_m(u_ij), B = sum_j g_ij^2,
g scaled by sqrt(0.5) via the host-marshaled c rows.

All index gathers (neighbor positions, c_table rows) happen on the
host; the device kernel is a pure DVE pipeline working full-width over
all S*M = 200 pairs per partition.  v5 notes (from v3/v4 traces):
 - GPSIMD shares its SBUF port with the DVE, so concurrent GPSIMD
   elementwise work halves DVE throughput: everything runs on V.
 - the 3.3MB cj DMA is gated behind a dummy 1-element copy that
   depends on posj, so the small geometry inputs get the full fabric
   and V starts ~10us earlier.
 - Chebyshev recurrence, Y scratch, and g^2 run in bf16 (2x DVE modes);
   numpy emulation puts the accuracy cost at ~1e-3.
 - strided bf16 *writes* are ~4x slow; every op writes contiguously.
"""
import os
import sys

sys.path.insert(0, "/opt/trn_rl_repo")
os.environ.setdefault("NEURON_RT_RESET_CORES", "1")

import math
import numpy as np

from concourse import bacc, bass, mybir, tile
from concourse.bass_utils import run_bass_kernel_spmd

# problem constants
N_ATOMS = 10000
M_NBR = 20
N_TYPES = 4
N_DESC = 8
K_MAX = 8
L_MAX = 4
R_C = 5.0

NCORES = 8
P = 128
S = 10                      # atom slots per partition
CA = P * S                  # atoms per core = 1280
NTOT = NCORES * CA          # padded atom count = 10240
PAIRS = S * M_NBR           # 200 pairs per partition

F32 = mybir.dt.float32
BF16 = mybir.dt.bfloat16

SQ3 = math.sqrt(3.0)
C31 = math.sqrt(3.0 / 8.0)
C32 = math.sqrt(15.0)
C33 = math.sqrt(5.0 / 8.0)
SHELL_OFF = [0, 1, 4, 9, 16]

AF = mybir.ActivationFunctionType
ALU = mybir.AluOpType
AX = mybir.AxisListType


def _ap(t, off, dims):
    base = t[:]
    ap = [list(base.ap[0])] + [[s, c] for (s, c) in dims]
    return bass.AP(base.tensor, base.offset + off, ap)


def build_nc(debug=False):
    nc = bacc.Bacc()
    dxz_d = nc.declare_dram_parameter("dxz", [P, PAIRS * 3], BF16,
                                      isOutput=False)
    cj_d = nc.declare_dram_parameter("cj", [P, PAIRS * 64], BF16,
                                     isOutput=False)
    out_d = nc.declare_dram_parameter("out", [P, S * N_DESC * L_MAX], F32,
                                      isOutput=True)

    with tile.TileContext(nc) as tc:
        with tc.tile_pool(name="main", bufs=1) as pool:
            # activation biases must be const APs
            cvals = [math.pi / 2, 0.0]
            consts = pool.tile([P, len(cvals)], F32)
            for ci, cv in enumerate(cvals):
                nc.vector.memset(consts[:, ci:ci + 1], cv)
                nc.const_aps.aps[(F32, cv)] = consts[:, ci:ci + 1]

            ph1 = tc.tile_pool(name="ph1", bufs=1)
            cp = ph1.__enter__()

            dxyz = cp.tile([P, PAIRS * 3], BF16)   # [s, 3, j] rel. pos.
            nc.sync.dma_start(out=dxyz[:], in_=dxz_d[:])
            cj = cp.tile([P, PAIRS * 64], BF16)    # [k, s, d, j]
            # gate the big cj DMA behind dxyz via a WAW dependency: the
            # dummy copy reads dxyz, writes one element of cj, and the
            # DMA (which overwrites all of cj) must follow it.
            with nc.allow_low_precision(reason="dma gate dummy"):
                nc.vector.tensor_copy(out=cj[:, 0:1], in_=dxyz[:, 0:1])
            nc.scalar.dma_start(out=cj[:], in_=cj_d[:])

            # early memsets while waiting for posj
            Y = pool.tile([P, PAIRS * 16], BF16)   # [s, m, j]

            def y_slice(m, cnt=1):
                if cnt == 1:
                    return _ap(Y, m * M_NBR, [(16 * M_NBR, S), (1, M_NBR)])
                return _ap(Y, m * M_NBR,
                           [(16 * M_NBR, S), (M_NBR, cnt), (1, M_NBR)])

            Tall = cp.tile([P, K_MAX * PAIRS], BF16)   # [k][s,j]

            def t_slice(k):
                return _ap(Tall, k * PAIRS, [(1, PAIRS)])

            with nc.allow_low_precision(reason="bf16 basis"):
                nc.vector.memset(y_slice(0), 1.0)
                nc.vector.memset(t_slice(0), 1.0)

            # ---- geometry (f32): dxyz, r2, r, 1/r -----------------------
            C3 = PAIRS * 3
            SJ3 = 3 * M_NBR

            def c_slice(t, c):
                return _ap(t, c * M_NBR, [(SJ3, S), (1, M_NBR)])

            sq = cp.tile([P, C3], BF16)
            r2 = cp.tile([P, PAIRS], BF16)
            with nc.allow_low_precision(reason="bf16 geometry"):
                nc.vector.tensor_tensor(out=sq[:], in0=dxyz[:], in1=dxyz[:],
                                        op=ALU.mult)
                nc.vector.tensor_tensor(out=r2[:], in0=c_slice(sq, 0),
                                        in1=c_slice(sq, 1), op=ALU.add)
                nc.vector.tensor_tensor(
                    out=r2[:],
                    in0=_ap(r2, 0, [(M_NBR, S), (1, M_NBR)]),
                    in1=c_slice(sq, 2), op=ALU.add)
            # clamp r^2 at R_C^2: fc(R_C) = 0 exactly, so pairs beyond
            # the cutoff contribute 0 through fch regardless of u/tm1.
            with nc.allow_low_precision(reason="bf16 geometry"):
                nc.vector.tensor_scalar_min(out=r2[:], in0=r2[:],
                                            scalar1=R_C * R_C)
            rr = cp.tile([P, PAIRS], F32)
            nc.scalar.sqrt(out=rr[:], in_=r2[:])
            cosx = cp.tile([P, PAIRS], F32)
            nc.scalar.activation(out=cosx[:], in_=rr[:], func=AF.Sin,
                                 bias=math.pi / 2, scale=-math.pi / R_C)
            rinv = cp.tile([P, PAIRS], BF16)
            with nc.allow_low_precision(reason="bf16 reciprocal"):
                nc.vector.reciprocal(out=rinv[:], in_=rr[:])
            u = cp.tile([P, C3], BF16)             # [s, 3, j] unit vectors
            with nc.allow_low_precision(reason="bf16 unit vectors"):
                nc.vector.tensor_tensor(
                    out=u[:], in0=dxyz[:],
                    in1=_ap(rinv, 0, [(M_NBR, S), (0, 3), (1, M_NBR)]),
                    op=ALU.mult)

            # ---- Chebyshev recurrence in bf16 [k][s,j] ------------------
            tm1 = cp.tile([P, PAIRS], F32)
            nc.vector.tensor_scalar(out=tm1[:], in0=rr[:], scalar1=1.0 / R_C,
                                    scalar2=-1.0, op0=ALU.mult, op1=ALU.add)
            x2 = cp.tile([P, PAIRS], BF16)
            with nc.allow_low_precision(reason="bf16 chebyshev"):
                # T1 = 2*tm1^2 - 1 written into Tall[1]
                nc.vector.scalar_tensor_tensor(out=t_slice(1), in0=tm1[:],
                                               scalar=2.0, in1=tm1[:],
                                               op0=ALU.mult, op1=ALU.mult)
                nc.vector.tensor_scalar(out=t_slice(1), in0=t_slice(1),
                                        scalar1=-1.0, scalar2=None,
                                        op0=ALU.add)
                nc.vector.tensor_scalar_mul(out=x2[:], in0=t_slice(1),
                                            scalar1=2.0)
                for k in range(2, K_MAX):
                    nc.vector.tensor_tensor(out=t_slice(k), in0=x2[:],
                                            in1=t_slice(k - 1), op=ALU.mult)
                    if k == 2:
                        nc.vector.tensor_scalar(out=t_slice(k),
                                                in0=t_slice(k), scalar1=-1.0,
                                                scalar2=None, op0=ALU.add)
                    else:
                        nc.vector.tensor_tensor(out=t_slice(k),
                                                in0=t_slice(k),
                                                in1=t_slice(k - 2),
                                                op=ALU.subtract)

            # fch = 0.25*cosx + 0.25  (= 0.5 * fc), bf16
            fch = cp.tile([P, PAIRS], BF16)
            with nc.allow_low_precision(reason="bf16 fch"):
                nc.vector.tensor_scalar(out=fch[:], in0=cosx[:], scalar1=0.25,
                                        scalar2=0.25, op0=ALU.mult,
                                        op1=ALU.add)

            # ---- radial basis f[k][s,j] = Tall[k] * fch -----------------
            # k-major like Tall, so this op is fully contiguous (2x mode).
            # (k=0 slice of Tall is 1, so f[0] = fch; the "+1" of the
            # reference basis is folded into c column 0 on the host)
            f = pool.tile([P, K_MAX * PAIRS], BF16)
            with nc.allow_low_precision(reason="bf16 radial basis"):
                nc.vector.tensor_tensor(
                    out=f[:], in0=Tall[:],
                    in1=_ap(fch, 0, [(0, K_MAX), (1, PAIRS)]),
                    op=ALU.mult)

            # ---- spherical harmonics Y[s,m,j] (bf16) --------------------
            # within-shell m order: l=2 -> [xy, xz, yz, z2, x2-y2]
            #                       l=3 -> [z3, x(5z2-1), y(5z2-1),
            #                               (x2-y2)z, xyz, tl5*x, tl6*y]
            def u_c(c, cnt=1):
                if cnt == 1:
                    return c_slice(u, c)
                return _ap(u, c * M_NBR, [(SJ3, S), (M_NBR, cnt), (1, M_NBR)])

            sc = cp.tile([P, PAIRS * 6], BF16)
            zs = cp.tile([P, PAIRS], BF16)

            def sc_t(i, cnt=1):
                if cnt == 1:
                    return _ap(sc, i * PAIRS, [(1, PAIRS)])
                return _ap(sc, i * PAIRS, [(M_NBR, S), (PAIRS, cnt),
                                           (1, M_NBR)])

            x2c, y2c, z2c, xyc, dxyc, tl = [sc_t(i) for i in range(6)]
            with nc.allow_low_precision(reason="bf16 Y"):
                nc.vector.tensor_copy(
                    out=y_slice(1, 3),
                    in_=_ap(u, 0, [(SJ3, S), (M_NBR, 3), (1, M_NBR)]))
                # x2c, y2c, z2c in one op
                nc.vector.tensor_tensor(out=sc_t(0, 3), in0=u_c(0, 3),
                                        in1=u_c(0, 3), op=ALU.mult)
                nc.vector.tensor_tensor(out=xyc, in0=u_c(0), in1=u_c(1),
                                        op=ALU.mult)
                nc.vector.tensor_scalar_mul(out=y_slice(4), in0=xyc,
                                            scalar1=SQ3)
                # Y5 = sqrt3*uz*ux, Y6 = sqrt3*uz*uy via pre-scaled uz
                # (STT is limited to 2 free dims; TT allows the pair)
                nc.vector.tensor_scalar_mul(out=zs[:], in0=u_c(2),
                                            scalar1=SQ3)
                nc.vector.tensor_tensor(
                    out=y_slice(5, 2),
                    in0=_ap(zs, 0, [(M_NBR, S), (0, 2), (1, M_NBR)]),
                    in1=u_c(0, 2), op=ALU.mult)
                nc.vector.tensor_scalar(out=y_slice(7), in0=z2c, scalar1=1.5,
                                        scalar2=-0.5, op0=ALU.mult,
                                        op1=ALU.add)
                nc.vector.tensor_tensor(out=dxyc, in0=x2c, in1=y2c,
                                        op=ALU.subtract)
                nc.vector.tensor_scalar_mul(out=y_slice(8), in0=dxyc,
                                            scalar1=SQ3 / 2)
                nc.vector.tensor_scalar(out=tl, in0=z2c, scalar1=2.5,
                                        scalar2=-1.5, op0=ALU.mult,
                                        op1=ALU.add)
                nc.vector.tensor_tensor(out=y_slice(9), in0=tl, in1=u_c(2),
                                        op=ALU.mult)
                nc.vector.tensor_scalar(out=tl, in0=z2c, scalar1=5.0 * C31,
                                        scalar2=-C31, op0=ALU.mult,
                                        op1=ALU.add)
                # Y10 = tl*ux, Y11 = tl*uy in one op
                nc.vector.tensor_tensor(
                    out=y_slice(10, 2),
                    in0=_ap(sc, 5 * PAIRS, [(M_NBR, S), (0, 2), (1, M_NBR)]),
                    in1=u_c(0, 2), op=ALU.mult)
                nc.vector.scalar_tensor_tensor(out=y_slice(12), in0=dxyc,
                                               scalar=C32 / 2, in1=u_c(2),
                                               op0=ALU.mult, op1=ALU.mult)
                nc.vector.scalar_tensor_tensor(out=y_slice(13), in0=xyc,
                                               scalar=C32, in1=u_c(2),
                                               op0=ALU.mult, op1=ALU.mult)
                nc.vector.scalar_tensor_tensor(out=tl, in0=y2c, scalar=3.0,
                                               in1=x2c, op0=ALU.mult,
                                               op1=ALU.subtract)
                nc.vector.scalar_tensor_tensor(out=y_slice(14), in0=tl,
                                               scalar=-C33, in1=u_c(0),
                                               op0=ALU.mult, op1=ALU.mult)
                nc.vector.scalar_tensor_tensor(out=tl, in0=x2c, scalar=3.0,
                                               in1=y2c, op0=ALU.mult,
                                               op1=ALU.subtract)
                nc.vector.scalar_tensor_tensor(out=y_slice(15), in0=tl,
                                               scalar=C33, in1=u_c(1),
                                               op0=ALU.mult, op1=ALU.mult)

            # ---- g[s,d,j] = sum_k cj[k,s,d,j] * f[k,s,j] ----------------
            # cj is host-marshaled k-major so the whole k tree is flat
            # contiguous adds (2x mode) and g lands in [s,d,j] directly.
            x2t = cp.tile([P, PAIRS * 64], BF16)     # [k, s, d, j]
            a4 = cp.tile([P, PAIRS * 32], BF16)      # [k4, s, d, j]
            a2 = cp.tile([P, PAIRS * 16], BF16)      # [k2, s, d, j]
            g = pool.tile([P, PAIRS * N_DESC], BF16)   # [s, d, j]
            with nc.allow_low_precision(reason="bf16 contraction"):
                nc.vector.tensor_tensor(
                    out=_ap(x2t, 0, [(1600, K_MAX), (160, S), (20, N_DESC),
                                     (1, M_NBR)]),
                    in0=_ap(cj, 0, [(1600, K_MAX), (160, S), (20, N_DESC),
                                    (1, M_NBR)]),
                    in1=_ap(f, 0, [(PAIRS, K_MAX), (M_NBR, S), (0, N_DESC),
                                   (1, M_NBR)]),
                    op=ALU.mult)
                nc.vector.tensor_tensor(
                    out=a4[:], in0=x2t[:, 0:6400], in1=x2t[:, 6400:12800],
                    op=ALU.add)
                nc.vector.tensor_tensor(
                    out=a2[:], in0=a4[:, 0:3200], in1=a4[:, 3200:6400],
                    op=ALU.add)
                nc.vector.tensor_tensor(
                    out=g[:], in0=a2[:, 0:1600], in1=a2[:, 1600:3200],
                    op=ALU.add)

            ph1.__exit__(None, None, None)
            ph2 = tc.tile_pool(name="ph2", bufs=1)
            p2 = ph2.__enter__()

            # ---- A[s,d,m] = sum_j g * Y  (tree over j = 8+8+4) ----------
            # m has 17 slots: 16 spherical harmonics plus g^2 (so the
            # tree also produces B[s,d] = A[s,d,16] for free).
            MM = 17
            xa = p2.tile([P, S * N_DESC * MM * M_NBR], BF16)  # [s,d,m,j]
            t8 = p2.tile([P, S * N_DESC * MM * 8], BF16)
            t4 = p2.tile([P, S * N_DESC * MM * 4], BF16)
            t4b = p2.tile([P, S * N_DESC * MM * 4], BF16)
            t2 = p2.tile([P, S * N_DESC * MM * 2], BF16)
            A = pool.tile([P, S * N_DESC * MM], F32)          # [s, d, m17]
            SX, DX = N_DESC * MM * M_NBR, MM * M_NBR
            with nc.allow_low_precision(reason="bf16 outer product"):
                # g^2 into m=16 (contiguous both sides -> 2x mode)
                nc.vector.tensor_tensor(
                    out=_ap(xa, 16 * M_NBR, [(SX, S), (DX, N_DESC),
                                             (1, M_NBR)]),
                    in0=_ap(g, 0, [(160, S), (20, N_DESC), (1, M_NBR)]),
                    in1=_ap(g, 0, [(160, S), (20, N_DESC), (1, M_NBR)]),
                    op=ALU.mult)
                nc.vector.tensor_tensor(
                    out=_ap(xa, 0, [(SX, S), (DX, N_DESC), (20, 16),
                                    (1, 20)]),
                    in0=_ap(g, 0, [(160, S), (20, N_DESC), (0, 16), (1, 20)]),
                    in1=_ap(Y, 0, [(320, S), (0, N_DESC), (20, 16), (1, 20)]),
                    op=ALU.mult)
                nc.vector.tensor_tensor(
                    out=_ap(t8, 0, [(MM * 64, S), (MM * 8, N_DESC), (8, MM),
                                    (1, 8)]),
                    in0=_ap(xa, 0, [(SX, S), (DX, N_DESC), (20, MM),
                                    (1, 8)]),
                    in1=_ap(xa, 8, [(SX, S), (DX, N_DESC), (20, MM),
                                    (1, 8)]),
                    op=ALU.add)
                nc.vector.tensor_tensor(
                    out=_ap(t4, 0, [(MM * 32, S), (MM * 4, N_DESC), (4, MM),
                                    (1, 4)]),
                    in0=_ap(t8, 0, [(MM * 64, S), (MM * 8, N_DESC), (8, MM),
                                    (1, 4)]),
                    in1=_ap(t8, 4, [(MM * 64, S), (MM * 8, N_DESC), (8, MM),
                                    (1, 4)]),
                    op=ALU.add)
                nc.vector.tensor_tensor(
                    out=_ap(t4b, 0, [(MM * 32, S), (MM * 4, N_DESC), (4, MM),
                                     (1, 4)]),
                    in0=_ap(t4, 0, [(MM * 32, S), (MM * 4, N_DESC), (4, MM),
                                    (1, 4)]),
                    in1=_ap(xa, 16, [(SX, S), (DX, N_DESC), (20, MM),
                                     (1, 4)]),
                    op=ALU.add)
                nc.vector.tensor_tensor(
                    out=_ap(t2, 0, [(MM * 16, S), (MM * 2, N_DESC), (2, MM),
                                    (1, 2)]),
                    in0=_ap(t4b, 0, [(MM * 32, S), (MM * 4, N_DESC), (4, MM),
                                     (1, 2)]),
                    in1=_ap(t4b, 2, [(MM * 32, S), (MM * 4, N_DESC), (4, MM),
                                     (1, 2)]),
                    op=ALU.add)
            nc.vector.tensor_tensor(
                out=_ap(A, 0, [(N_DESC * MM, S), (MM, N_DESC), (1, MM)]),
                in0=_ap(t2, 0, [(MM * 16, S), (MM * 2, N_DESC), (2, MM)]),
                in1=_ap(t2, 1, [(MM * 16, S), (MM * 2, N_DESC), (2, MM)]),
                op=ALU.add)

            if debug:
                for nm, t, dt in [("f", f, BF16), ("Y", Y, BF16),
                                  ("g", g, BF16), ("A", A, F32)]:
                    dd = nc.declare_dram_parameter(
                        "d_" + nm, [P, t.shape[1]], dt, isOutput=True)
                    nc.sync.dma_start(out=dd[:], in_=t[:])

            # ---- q[s,d,l] = sum_{m in shell l} A^2 - B ------------------
            # two atom-halves so the first half's output DMA overlaps the
            # second half's compute
            Asq = pool.tile([P, S * N_DESC * 16], F32)
            outq = pool.tile([P, S * N_DESC * L_MAX], F32)
            q2la = pool.tile([P, S * N_DESC * L_MAX], F32)
            SH_ = S // 2
            for h in range(2):
                sqo, ao, qo = h * SH_ * 128, h * SH_ * N_DESC * MM,                     h * SH_ * N_DESC * L_MAX
                nc.vector.tensor_tensor(
                    out=_ap(Asq, sqo, [(128, SH_), (16, N_DESC), (1, 16)]),
                    in0=_ap(A, ao, [(N_DESC * MM, SH_), (MM, N_DESC),
                                    (1, 16)]),
                    in1=_ap(A, ao, [(N_DESC * MM, SH_), (MM, N_DESC),
                                    (1, 16)]),
                    op=ALU.mult)
                for l in range(L_MAX):
                    cnt = SHELL_OFF[l + 1] - SHELL_OFF[l]
                    nc.vector.tensor_reduce(
                        out=_ap(q2la, qo + l, [(N_DESC * L_MAX, SH_),
                                               (L_MAX, N_DESC)]),
                        in_=_ap(Asq, sqo + SHELL_OFF[l],
                                [(N_DESC * 16, SH_), (16, N_DESC), (1, cnt)]),
                        axis=AX.X, op=ALU.add)
                nc.vector.tensor_tensor(
                    out=_ap(outq, qo, [(1, SH_ * N_DESC * L_MAX)]),
                    in0=_ap(q2la, qo, [(1, SH_ * N_DESC * L_MAX)]),
                    in1=_ap(A, ao + 16, [(N_DESC * MM, SH_), (MM, N_DESC),
                                         (0, L_MAX)]),
                    op=ALU.subtract)
                nq = SH_ * N_DESC * L_MAX
                nc.sync.dma_start(out=out_d[:, qo:qo + nq],
                                  in_=outq[:, qo:qo + nq])
            ph2.__exit__(None, None, None)
    nc.finalize()
    return nc


def make_inputs(types, positions, angular_neighbors, c_table):
    types = np.asarray(types).astype(np.int64)
    positions = np.ascontiguousarray(np.asarray(positions, dtype=np.float32))
    nbr = np.asarray(angular_neighbors).astype(np.int64)
    c_table = np.asarray(c_table, dtype=np.float32)
    import ml_dtypes

    pad = NTOT - N_ATOMS
    types_pad = np.concatenate([types, np.repeat(types[-1:], pad, 0)], 0)
    pos_pad = np.concatenate([positions, np.repeat(positions[-1:], pad, 0)],
                             0)
    nbr_pad = np.concatenate([nbr, np.repeat(nbr[-1:], pad, 0)], 0)

    # per-(t_i,t_j) c rows in [d, k] order, sqrt(0.5)-scaled.
    # Column 0 absorbs sum_k c[d,k] (device basis is [fch, T_1*fch, ...]).
    c_adj = c_table.astype(np.float64).copy()
    c_adj[..., 0] += c_table.astype(np.float64).sum(-1)
    c16 = (c_adj * math.sqrt(0.5)).astype(ml_dtypes.bfloat16)  # [4,4,8,8]

    pvec = np.arange(P)
    svec = np.arange(S)
    in_maps = []
    for c in range(NCORES):
        atom = c * CA + pvec[:, None] * S + svec[None, :]       # [P, S]
        nbrs = nbr_pad[atom]                                    # [P, S, 20]
        # relative positions, component-major [P, S, 3, M], bf16
        dxz = (pos_pad[nbrs] - pos_pad[atom][:, :, None, :]).astype(
            np.float32)
        dxz = dxz.transpose(0, 1, 3, 2).reshape(P, PAIRS * 3)
        dxz = dxz.astype(ml_dtypes.bfloat16)
        tj = types_pad[nbrs]                                    # [P, S, 20]
        # [P,S,M,d,k] -> k-major [P,k,S,d,M(j)]
        cjf = c16[types_pad[atom][:, :, None], tj]
        cj = cjf.transpose(0, 4, 1, 3, 2).reshape(P, PAIRS * 64)
        in_maps.append({
            "dxz": np.ascontiguousarray(dxz),
            "cj": np.ascontiguousarray(cj),
        })
    return in_maps


_NC_CACHE = None


def kernel(types, positions, angular_neighbors, c_table):
    global _NC_CACHE
    in_maps = make_inputs(types, positions, angular_neighbors, c_table)
    if _NC_CACHE is None:
        _NC_CACHE = build_nc()
    res = run_bass_kernel_spmd(_NC_CACHE, in_maps,
                               core_ids=list(range(NCORES)))
    outs = [res.results[c]["out"].reshape(CA, N_DESC, L_MAX)
            for c in range(NCORES)]
    q = np.concatenate(outs, 0)[:N_ATOMS]
    return np.ascontiguousarray(q.astype(np.float32))


if __name__ == "__main__":
    z = np.load("/tmp/ref_cache.npz")
    inputs = {k: z[k] for k in
              ("types", "positions", "angular_neighbors", "c_table")}
    exp = z["exp"]
    act = kernel(**inputs)
    rel = np.linalg.norm(act - exp) / np.linalg.norm(exp)
    print("Relative error:", rel)


# revision 19
# speedup vs baseline: 1.0376x; 1.0275x over previous
"""Trainium2 Bass kernel v5 for nn_AngularDescriptor (gnn_message_passing).

Legendre-addition-theorem factorization: q[i,d,l] = sum_{m in shell l}
A[i,d,m]^2 - B[i,d] with A = sum_j g_ij Y_m(u_ij), B = sum_j g_ij^2,
g scaled by sqrt(0.5) via the host-marshaled c rows.

All index gathers (neighbor positions, c_table rows) happen on the
host; the device kernel is a pure DVE pipeline working full-width over
all S*M = 200 pairs per partition.  v5 notes (from v3/v4 traces):
 - GPSIMD shares its SBUF port with the DVE, so concurrent GPSIMD
   elementwise work halves DVE throughput: everything runs on V.
 - the 3.3MB cj DMA is gated behind a dummy 1-element copy that
   depends on posj, so the small geometry inputs get the full fabric
   and V starts ~10us earlier.
 - Chebyshev recurrence, Y scratch, and g^2 run in bf16 (2x DVE modes);
   numpy emulation puts the accuracy cost at ~1e-3.
 - strided bf16 *writes* are ~4x slow; every op writes contiguously.
"""
import os
import sys

sys.path.insert(0, "/opt/trn_rl_repo")
os.environ.setdefault("NEURON_RT_RESET_CORES", "1")

import math
import numpy as np

from concourse import bacc, bass, mybir, tile
from concourse.bass_utils import run_bass_kernel_spmd

# problem constants
N_ATOMS = 10000
M_NBR = 20
N_TYPES = 4
N_DESC = 8
K_MAX = 8
L_MAX = 4
R_C = 5.0

NCORES = 8
P = 128
S = 10                      # atom slots per partition
CA = P * S                  # atoms per core = 1280
NTOT = NCORES * CA          # padded atom count = 10240
PAIRS = S * M_NBR           # 200 pairs per partition

F32 = mybir.dt.float32
BF16 = mybir.dt.bfloat16

SQ3 = math.sqrt(3.0)
C31 = math.sqrt(3.0 / 8.0)
C32 = math.sqrt(15.0)
C33 = math.sqrt(5.0 / 8.0)
SHELL_OFF = [0, 1, 4, 9, 16]

AF = mybir.ActivationFunctionType
ALU = mybir.AluOpType
AX = mybir.AxisListType


def _ap(t, off, dims):
    base = t[:]
    ap = [list(base.ap[0])] + [[s, c] for (s, c) in dims]
    return bass.AP(base.tensor, base.offset + off, ap)


def build_nc(debug=False):
    nc = bacc.Bacc()
    dxz_d = nc.declare_dram_parameter("dxz", [P, PAIRS * 3], BF16,
                                      isOutput=False)
    cj_d = nc.declare_dram_parameter("cj", [P, PAIRS * 64], BF16,
                                     isOutput=False)
    out_d = nc.declare_dram_parameter("out", [P, S * N_DESC * L_MAX], F32,
                                      isOutput=True)

    with tile.TileContext(nc) as tc:
        with tc.tile_pool(name="main", bufs=1) as pool:
            # activation biases must be const APs
            cvals = [math.pi / 2, 0.0]
            consts = pool.tile([P, len(cvals)], F32)
            for ci, cv in enumerate(cvals):
                nc.vector.memset(consts[:, ci:ci + 1], cv)
                nc.const_aps.aps[(F32, cv)] = consts[:, ci:ci + 1]

            ph1 = tc.tile_pool(name="ph1", bufs=1)
            cp = ph1.__enter__()

            dxyz = cp.tile([P, PAIRS * 3], BF16)   # [s, 3, j] rel. pos.
            nc.sync.dma_start(out=dxyz[:], in_=dxz_d[:])
            cj = cp.tile([P, PAIRS * 64], BF16)    # [k, s, d, j]
            # gate the big cj DMA behind dxyz via a WAW dependency: the
            # dummy copy reads dxyz, writes one element of cj, and the
            # DMA (which overwrites all of cj) must follow it.
            with nc.allow_low_precision(reason="dma gate dummy"):
                nc.vector.tensor_copy(out=cj[:, 0:1], in_=dxyz[:, 0:1])
            nc.scalar.dma_start(out=cj[:], in_=cj_d[:])

            # early memsets while waiting for posj
            Y = pool.tile([P, PAIRS * 16], BF16)   # [s, m, j]

            def y_slice(m, cnt=1):
                if cnt == 1:
                    return _ap(Y, m * M_NBR, [(16 * M_NBR, S), (1, M_NBR)])
                return _ap(Y, m * M_NBR,
                           [(16 * M_NBR, S), (M_NBR, cnt), (1, M_NBR)])

            Tall = cp.tile([P, K_MAX * PAIRS], BF16)   # [k][s,j]

            def t_slice(k):
                return _ap(Tall, k * PAIRS, [(1, PAIRS)])

            with nc.allow_low_precision(reason="bf16 basis"):
                nc.vector.memset(y_slice(0), 1.0)
                nc.vector.memset(t_slice(0), 1.0)

            # ---- geometry (f32): dxyz, r2, r, 1/r -----------------------
            C3 = PAIRS * 3
            SJ3 = 3 * M_NBR

            def c_slice(t, c):
                return _ap(t, c * M_NBR, [(SJ3, S), (1, M_NBR)])

            sq = cp.tile([P, C3], BF16)
            r2 = cp.tile([P, PAIRS], BF16)
            with nc.allow_low_precision(reason="bf16 geometry"):
                nc.vector.tensor_tensor(out=sq[:], in0=dxyz[:], in1=dxyz[:],
                                        op=ALU.mult)
                nc.vector.tensor_tensor(out=r2[:], in0=c_slice(sq, 0),
                                        in1=c_slice(sq, 1), op=ALU.add)
                nc.vector.tensor_tensor(
                    out=r2[:],
                    in0=_ap(r2, 0, [(M_NBR, S), (1, M_NBR)]),
                    in1=c_slice(sq, 2), op=ALU.add)
            # clamp r^2 at R_C^2: fc(R_C) = 0 exactly, so pairs beyond
            # the cutoff contribute 0 through fch regardless of u/tm1.
            with nc.allow_low_precision(reason="bf16 geometry"):
                nc.vector.tensor_scalar_min(out=r2[:], in0=r2[:],
                                            scalar1=R_C * R_C)
            rr = cp.tile([P, PAIRS], F32)
            nc.scalar.sqrt(out=rr[:], in_=r2[:])
            cosx = cp.tile([P, PAIRS], F32)
            nc.scalar.activation(out=cosx[:], in_=rr[:], func=AF.Sin,
                                 bias=math.pi / 2, scale=-math.pi / R_C)
            rinv = cp.tile([P, PAIRS], BF16)
            with nc.allow_low_precision(reason="bf16 reciprocal"):
                nc.vector.reciprocal(out=rinv[:], in_=rr[:])
            u = cp.tile([P, C3], BF16)             # [s, 3, j] unit vectors
            with nc.allow_low_precision(reason="bf16 unit vectors"):
                nc.vector.tensor_tensor(
                    out=u[:], in0=dxyz[:],
                    in1=_ap(rinv, 0, [(M_NBR, S), (0, 3), (1, M_NBR)]),
                    op=ALU.mult)

            # ---- Chebyshev recurrence in bf16 [k][s,j] ------------------
            tm1 = cp.tile([P, PAIRS], F32)
            nc.vector.tensor_scalar(out=tm1[:], in0=rr[:], scalar1=1.0 / R_C,
                                    scalar2=-1.0, op0=ALU.mult, op1=ALU.add)
            x2 = cp.tile([P, PAIRS], BF16)
            tsq = cp.tile([P, PAIRS], BF16)
            with nc.allow_low_precision(reason="bf16 chebyshev"):
                # T1 = 2*tm1^2 - 1; x2 = 2*T1 = 4*tm1^2 - 2 (from tsq,
                # independent of the T1 write for better ILP)
                nc.vector.tensor_tensor(out=tsq[:], in0=tm1[:], in1=tm1[:],
                                        op=ALU.mult)
                nc.vector.tensor_scalar(out=t_slice(1), in0=tsq[:],
                                        scalar1=2.0, scalar2=-1.0,
                                        op0=ALU.mult, op1=ALU.add)
                nc.vector.tensor_scalar(out=x2[:], in0=tsq[:],
                                        scalar1=4.0, scalar2=-2.0,
                                        op0=ALU.mult, op1=ALU.add)
                for k in range(2, K_MAX):
                    nc.vector.tensor_tensor(out=t_slice(k), in0=x2[:],
                                            in1=t_slice(k - 1), op=ALU.mult)
                    if k == 2:
                        nc.vector.tensor_scalar(out=t_slice(k),
                                                in0=t_slice(k), scalar1=-1.0,
                                                scalar2=None, op0=ALU.add)
                    else:
                        nc.vector.tensor_tensor(out=t_slice(k),
                                                in0=t_slice(k),
                                                in1=t_slice(k - 2),
                                                op=ALU.subtract)

            # fch = 0.25*cosx + 0.25  (= 0.5 * fc), bf16
            fch = cp.tile([P, PAIRS], BF16)
            with nc.allow_low_precision(reason="bf16 fch"):
                nc.vector.tensor_scalar(out=fch[:], in0=cosx[:], scalar1=0.25,
                                        scalar2=0.25, op0=ALU.mult,
                                        op1=ALU.add)

            # ---- radial basis f[k][s,j] = Tall[k] * fch -----------------
            # k-major like Tall, so this op is fully contiguous (2x mode).
            # (k=0 slice of Tall is 1, so f[0] = fch; the "+1" of the
            # reference basis is folded into c column 0 on the host)
            f = pool.tile([P, K_MAX * PAIRS], BF16)
            with nc.allow_low_precision(reason="bf16 radial basis"):
                nc.vector.tensor_tensor(
                    out=f[:], in0=Tall[:],
                    in1=_ap(fch, 0, [(0, K_MAX), (1, PAIRS)]),
                    op=ALU.mult)

            # ---- spherical harmonics Y[s,m,j] (bf16) --------------------
            # within-shell m order: l=2 -> [xy, xz, yz, z2, x2-y2]
            #                       l=3 -> [z3, x(5z2-1), y(5z2-1),
            #                               (x2-y2)z, xyz, tl5*x, tl6*y]
            def u_c(c, cnt=1):
                if cnt == 1:
                    return c_slice(u, c)
                return _ap(u, c * M_NBR, [(SJ3, S), (M_NBR, cnt), (1, M_NBR)])

            sc = cp.tile([P, PAIRS * 7], BF16)
            zs = cp.tile([P, PAIRS], BF16)

            def sc_t(i, cnt=1):
                if cnt == 1:
                    return _ap(sc, i * PAIRS, [(1, PAIRS)])
                return _ap(sc, i * PAIRS, [(M_NBR, S), (PAIRS, cnt),
                                           (1, M_NBR)])

            x2c, y2c, z2c, xyc, dxyc, tl = [sc_t(i) for i in range(6)]
            with nc.allow_low_precision(reason="bf16 Y"):
                nc.vector.tensor_copy(
                    out=y_slice(1, 3),
                    in_=_ap(u, 0, [(SJ3, S), (M_NBR, 3), (1, M_NBR)]))
                # x2c, y2c, z2c in one op
                nc.vector.tensor_tensor(out=sc_t(0, 3), in0=u_c(0, 3),
                                        in1=u_c(0, 3), op=ALU.mult)
                nc.vector.tensor_tensor(out=xyc, in0=u_c(0), in1=u_c(1),
                                        op=ALU.mult)
                nc.vector.tensor_scalar_mul(out=y_slice(4), in0=xyc,
                                            scalar1=SQ3)
                # Y5 = sqrt3*uz*ux, Y6 = sqrt3*uz*uy via pre-scaled uz
                # (STT is limited to 2 free dims; TT allows the pair)
                nc.vector.tensor_scalar_mul(out=zs[:], in0=u_c(2),
                                            scalar1=SQ3)
                nc.vector.tensor_tensor(
                    out=y_slice(5, 2),
                    in0=_ap(zs, 0, [(M_NBR, S), (0, 2), (1, M_NBR)]),
                    in1=u_c(0, 2), op=ALU.mult)
                nc.vector.tensor_scalar(out=y_slice(7), in0=z2c, scalar1=1.5,
                                        scalar2=-0.5, op0=ALU.mult,
                                        op1=ALU.add)
                nc.vector.tensor_tensor(out=dxyc, in0=x2c, in1=y2c,
                                        op=ALU.subtract)
                nc.vector.tensor_scalar_mul(out=y_slice(8), in0=dxyc,
                                            scalar1=SQ3 / 2)
                nc.vector.tensor_scalar(out=tl, in0=z2c, scalar1=2.5,
                                        scalar2=-1.5, op0=ALU.mult,
                                        op1=ALU.add)
                nc.vector.tensor_tensor(out=y_slice(9), in0=tl, in1=u_c(2),
                                        op=ALU.mult)
                nc.vector.tensor_scalar(out=tl, in0=z2c, scalar1=5.0 * C31,
                                        scalar2=-C31, op0=ALU.mult,
                                        op1=ALU.add)
                # Y10 = tl*ux, Y11 = tl*uy in one op
                nc.vector.tensor_tensor(
                    out=y_slice(10, 2),
                    in0=_ap(sc, 5 * PAIRS, [(M_NBR, S), (0, 2), (1, M_NBR)]),
                    in1=u_c(0, 2), op=ALU.mult)
                # Y12 = (C32/2*dxyc)*uz, Y13 = (C32*xyc)*uz: prescale
                # into adjacent scratch then one paired TT
                nc.vector.tensor_scalar_mul(out=sc_t(4), in0=dxyc,
                                            scalar1=C32 / 2)
                nc.vector.tensor_scalar_mul(out=sc_t(6), in0=xyc,
                                            scalar1=C32)
                nc.vector.tensor_tensor(
                    out=y_slice(12, 2),
                    in0=_ap(sc, 4 * PAIRS, [(M_NBR, S), (2 * PAIRS, 2),
                                            (1, M_NBR)]),
                    in1=_ap(u, 2 * M_NBR, [(SJ3, S), (0, 2), (1, M_NBR)]),
                    op=ALU.mult)
                nc.vector.scalar_tensor_tensor(out=tl, in0=y2c, scalar=3.0,
                                               in1=x2c, op0=ALU.mult,
                                               op1=ALU.subtract)
                nc.vector.scalar_tensor_tensor(out=y_slice(14), in0=tl,
                                               scalar=-C33, in1=u_c(0),
                                               op0=ALU.mult, op1=ALU.mult)
                nc.vector.scalar_tensor_tensor(out=tl, in0=x2c, scalar=3.0,
                                               in1=y2c, op0=ALU.mult,
                                               op1=ALU.subtract)
                nc.vector.scalar_tensor_tensor(out=y_slice(15), in0=tl,
                                               scalar=C33, in1=u_c(1),
                                               op0=ALU.mult, op1=ALU.mult)

            # ---- g[s,d,j] = sum_k cj[k,s,d,j] * f[k,s,j] ----------------
            # cj is host-marshaled k-major so the whole k tree is flat
            # contiguous adds (2x mode) and g lands in [s,d,j] directly.
            x2t = cp.tile([P, PAIRS * 64], BF16)     # [k, s, d, j]
            a4 = cp.tile([P, PAIRS * 32], BF16)      # [k4, s, d, j]
            a2 = cp.tile([P, PAIRS * 16], BF16)      # [k2, s, d, j]
            g = pool.tile([P, PAIRS * N_DESC], BF16)   # [s, d, j]
            with nc.allow_low_precision(reason="bf16 contraction"):
                nc.vector.tensor_tensor(
                    out=_ap(x2t, 0, [(1600, K_MAX), (160, S), (20, N_DESC),
                                     (1, M_NBR)]),
                    in0=_ap(cj, 0, [(1600, K_MAX), (160, S), (20, N_DESC),
                                    (1, M_NBR)]),
                    in1=_ap(f, 0, [(PAIRS, K_MAX), (M_NBR, S), (0, N_DESC),
                                   (1, M_NBR)]),
                    op=ALU.mult)
                nc.vector.tensor_tensor(
                    out=a4[:], in0=x2t[:, 0:6400], in1=x2t[:, 6400:12800],
                    op=ALU.add)
                nc.vector.tensor_tensor(
                    out=a2[:], in0=a4[:, 0:3200], in1=a4[:, 3200:6400],
                    op=ALU.add)
                nc.vector.tensor_tensor(
                    out=g[:], in0=a2[:, 0:1600], in1=a2[:, 1600:3200],
                    op=ALU.add)

            ph1.__exit__(None, None, None)
            ph2 = tc.tile_pool(name="ph2", bufs=1)
            p2 = ph2.__enter__()

            # ---- A[s,d,m] = sum_j g * Y  (tree over j = 8+8+4) ----------
            # m has 17 slots: 16 spherical harmonics plus g^2 (so the
            # tree also produces B[s,d] = A[s,d,16] for free).
            MM = 17
            xa = p2.tile([P, S * N_DESC * MM * M_NBR], BF16)  # [s,d,m,j]
            t8 = p2.tile([P, S * N_DESC * MM * 8], BF16)
            t4 = p2.tile([P, S * N_DESC * MM * 4], BF16)
            t4b = p2.tile([P, S * N_DESC * MM * 4], BF16)
            t2 = p2.tile([P, S * N_DESC * MM * 2], BF16)
            A = pool.tile([P, S * N_DESC * MM], F32)          # [s, d, m17]
            SX, DX = N_DESC * MM * M_NBR, MM * M_NBR
            with nc.allow_low_precision(reason="bf16 outer product"):
                # g^2 into m=16 (contiguous both sides -> 2x mode)
                nc.vector.tensor_tensor(
                    out=_ap(xa, 16 * M_NBR, [(SX, S), (DX, N_DESC),
                                             (1, M_NBR)]),
                    in0=_ap(g, 0, [(160, S), (20, N_DESC), (1, M_NBR)]),
                    in1=_ap(g, 0, [(160, S), (20, N_DESC), (1, M_NBR)]),
                    op=ALU.mult)
                nc.vector.tensor_tensor(
                    out=_ap(xa, 0, [(SX, S), (DX, N_DESC), (20, 16),
                                    (1, 20)]),
                    in0=_ap(g, 0, [(160, S), (20, N_DESC), (0, 16), (1, 20)]),
                    in1=_ap(Y, 0, [(320, S), (0, N_DESC), (20, 16), (1, 20)]),
                    op=ALU.mult)
                nc.vector.tensor_tensor(
                    out=_ap(t8, 0, [(MM * 64, S), (MM * 8, N_DESC), (8, MM),
                                    (1, 8)]),
                    in0=_ap(xa, 0, [(SX, S), (DX, N_DESC), (20, MM),
                                    (1, 8)]),
                    in1=_ap(xa, 8, [(SX, S), (DX, N_DESC), (20, MM),
                                    (1, 8)]),
                    op=ALU.add)
                nc.vector.tensor_tensor(
                    out=_ap(t4, 0, [(MM * 32, S), (MM * 4, N_DESC), (4, MM),
                                    (1, 4)]),
                    in0=_ap(t8, 0, [(MM * 64, S), (MM * 8, N_DESC), (8, MM),
                                    (1, 4)]),
                    in1=_ap(t8, 4, [(MM * 64, S), (MM * 8, N_DESC), (8, MM),
                                    (1, 4)]),
                    op=ALU.add)
                nc.vector.tensor_tensor(
                    out=_ap(t4b, 0, [(MM * 32, S), (MM * 4, N_DESC), (4, MM),
                                     (1, 4)]),
                    in0=_ap(t4, 0, [(MM * 32, S), (MM * 4, N_DESC), (4, MM),
                                    (1, 4)]),
                    in1=_ap(xa, 16, [(SX, S), (DX, N_DESC), (20, MM),
                                     (1, 4)]),
                    op=ALU.add)
                nc.vector.tensor_tensor(
                    out=_ap(t2, 0, [(MM * 16, S), (MM * 2, N_DESC), (2, MM),
                                    (1, 2)]),
                    in0=_ap(t4b, 0, [(MM * 32, S), (MM * 4, N_DESC), (4, MM),
                                     (1, 2)]),
                    in1=_ap(t4b, 2, [(MM * 32, S), (MM * 4, N_DESC), (4, MM),
                                     (1, 2)]),
                    op=ALU.add)
            nc.vector.tensor_tensor(
                out=_ap(A, 0, [(N_DESC * MM, S), (MM, N_DESC), (1, MM)]),
                in0=_ap(t2, 0, [(MM * 16, S), (MM * 2, N_DESC), (2, MM)]),
                in1=_ap(t2, 1, [(MM * 16, S), (MM * 2, N_DESC), (2, MM)]),
                op=ALU.add)

            if debug:
                for nm, t, dt in [("f", f, BF16), ("Y", Y, BF16),
                                  ("g", g, BF16), ("A", A, F32)]:
                    dd = nc.declare_dram_parameter(
                        "d_" + nm, [P, t.shape[1]], dt, isOutput=True)
                    nc.sync.dma_start(out=dd[:], in_=t[:])

            # ---- q[s,d,l] = sum_{m in shell l} A^2 - B ------------------
            # two atom-halves so the first half's output DMA overlaps the
            # second half's compute
            Asq = pool.tile([P, S * N_DESC * 16], F32)
            outq = pool.tile([P, S * N_DESC * L_MAX], F32)
            q2la = pool.tile([P, S * N_DESC * L_MAX], F32)
            SH_ = S // 2
            for h in range(2):
                sqo, ao, qo = h * SH_ * 128, h * SH_ * N_DESC * MM,                     h * SH_ * N_DESC * L_MAX
                nc.vector.tensor_tensor(
                    out=_ap(Asq, sqo, [(128, SH_), (16, N_DESC), (1, 16)]),
                    in0=_ap(A, ao, [(N_DESC * MM, SH_), (MM, N_DESC),
                                    (1, 16)]),
                    in1=_ap(A, ao, [(N_DESC * MM, SH_), (MM, N_DESC),
                                    (1, 16)]),
                    op=ALU.mult)
                for l in range(L_MAX):
                    cnt = SHELL_OFF[l + 1] - SHELL_OFF[l]
                    nc.vector.tensor_reduce(
                        out=_ap(q2la, qo + l, [(N_DESC * L_MAX, SH_),
                                               (L_MAX, N_DESC)]),
                        in_=_ap(Asq, sqo + SHELL_OFF[l],
                                [(N_DESC * 16, SH_), (16, N_DESC), (1, cnt)]),
                        axis=AX.X, op=ALU.add)
                nc.vector.tensor_tensor(
                    out=_ap(outq, qo, [(1, SH_ * N_DESC * L_MAX)]),
                    in0=_ap(q2la, qo, [(1, SH_ * N_DESC * L_MAX)]),
                    in1=_ap(A, ao + 16, [(N_DESC * MM, SH_), (MM, N_DESC),
                                         (0, L_MAX)]),
                    op=ALU.subtract)
                nq = SH_ * N_DESC * L_MAX
                nc.sync.dma_start(out=out_d[:, qo:qo + nq],
                                  in_=outq[:, qo:qo + nq])
            ph2.__exit__(None, None, None)
    nc.finalize()
    return nc


def make_inputs(types, positions, angular_neighbors, c_table):
    types = np.asarray(types).astype(np.int64)
    positions = np.ascontiguousarray(np.asarray(positions, dtype=np.float32))
    nbr = np.asarray(angular_neighbors).astype(np.int64)
    c_table = np.asarray(c_table, dtype=np.float32)
    import ml_dtypes

    pad = NTOT - N_ATOMS
    types_pad = np.concatenate([types, np.repeat(types[-1:], pad, 0)], 0)
    pos_pad = np.concatenate([positions, np.repeat(positions[-1:], pad, 0)],
                             0)
    nbr_pad = np.concatenate([nbr, np.repeat(nbr[-1:], pad, 0)], 0)

    # per-(t_i,t_j) c rows in [d, k] order, sqrt(0.5)-scaled.
    # Column 0 absorbs sum_k c[d,k] (device basis is [fch, T_1*fch, ...]).
    c_adj = c_table.astype(np.float64).copy()
    c_adj[..., 0] += c_table.astype(np.float64).sum(-1)
    c16 = (c_adj * math.sqrt(0.5)).astype(ml_dtypes.bfloat16)  # [4,4,8,8]

    pvec = np.arange(P)
    svec = np.arange(S)
    in_maps = []
    for c in range(NCORES):
        atom = c * CA + pvec[:, None] * S + svec[None, :]       # [P, S]
        nbrs = nbr_pad[atom]                                    # [P, S, 20]
        # relative positions, component-major [P, S, 3, M], bf16
        dxz = (pos_pad[nbrs] - pos_pad[atom][:, :, None, :]).astype(
            np.float32)
        dxz = dxz.transpose(0, 1, 3, 2).reshape(P, PAIRS * 3)
        dxz = dxz.astype(ml_dtypes.bfloat16)
        tj = types_pad[nbrs]                                    # [P, S, 20]
        # [P,S,M,d,k] -> k-major [P,k,S,d,M(j)]
        cjf = c16[types_pad[atom][:, :, None], tj]
        cj = cjf.transpose(0, 4, 1, 3, 2).reshape(P, PAIRS * 64)
        in_maps.append({
            "dxz": np.ascontiguousarray(dxz),
            "cj": np.ascontiguousarray(cj),
        })
    return in_maps


_NC_CACHE = None


def kernel(types, positions, angular_neighbors, c_table):
    global _NC_CACHE
    in_maps = make_inputs(types, positions, angular_neighbors, c_table)
    if _NC_CACHE is None:
        _NC_CACHE = build_nc()
    res = run_bass_kernel_spmd(_NC_CACHE, in_maps,
                               core_ids=list(range(NCORES)))
    outs = [res.results[c]["out"].reshape(CA, N_DESC, L_MAX)
            for c in range(NCORES)]
    q = np.concatenate(outs, 0)[:N_ATOMS]
    return np.ascontiguousarray(q.astype(np.float32))


if __name__ == "__main__":
    z = np.load("/tmp/ref_cache.npz")
    inputs = {k: z[k] for k in
              ("types", "positions", "angular_neighbors", "c_table")}
    exp = z["exp"]
    act = kernel(**inputs)
    rel = np.linalg.norm(act - exp) / np.linalg.norm(exp)
    print("Relative error:", rel)


# revision 20
# speedup vs baseline: 1.0404x; 1.0027x over previous
"""Trainium2 Bass kernel v5 for nn_AngularDescriptor (gnn_message_passing).

Legendre-addition-theorem factorization: q[i,d,l] = sum_{m in shell l}
A[i,d,m]^2 - B[i,d] with A = sum_j g_ij Y_m(u_ij), B = sum_j g_ij^2,
g scaled by sqrt(0.5) via the host-marshaled c rows.

All index gathers (neighbor positions, c_table rows) happen on the
host; the device kernel is a pure DVE pipeline working full-width over
all S*M = 200 pairs per partition.  v5 notes (from v3/v4 traces):
 - GPSIMD shares its SBUF port with the DVE, so concurrent GPSIMD
   elementwise work halves DVE throughput: everything runs on V.
 - the 3.3MB cj DMA is gated behind a dummy 1-element copy that
   depends on posj, so the small geometry inputs get the full fabric
   and V starts ~10us earlier.
 - Chebyshev recurrence, Y scratch, and g^2 run in bf16 (2x DVE modes);
   numpy emulation puts the accuracy cost at ~1e-3.
 - strided bf16 *writes* are ~4x slow; every op writes contiguously.
"""
import os
import sys

sys.path.insert(0, "/opt/trn_rl_repo")
os.environ.setdefault("NEURON_RT_RESET_CORES", "1")

import math
import numpy as np

from concourse import bacc, bass, mybir, tile
from concourse.bass_utils import run_bass_kernel_spmd

# problem constants
N_ATOMS = 10000
M_NBR = 20
N_TYPES = 4
N_DESC = 8
K_MAX = 8
L_MAX = 4
R_C = 5.0

NCORES = 8
P = 128
S = 10                      # atom slots per partition
CA = P * S                  # atoms per core = 1280
NTOT = NCORES * CA          # padded atom count = 10240
PAIRS = S * M_NBR           # 200 pairs per partition

F32 = mybir.dt.float32
BF16 = mybir.dt.bfloat16

SQ3 = math.sqrt(3.0)
C31 = math.sqrt(3.0 / 8.0)
C32 = math.sqrt(15.0)
C33 = math.sqrt(5.0 / 8.0)
SHELL_OFF = [0, 1, 4, 9, 16]

AF = mybir.ActivationFunctionType
ALU = mybir.AluOpType
AX = mybir.AxisListType


def _ap(t, off, dims):
    base = t[:]
    ap = [list(base.ap[0])] + [[s, c] for (s, c) in dims]
    return bass.AP(base.tensor, base.offset + off, ap)


def build_nc(debug=False):
    nc = bacc.Bacc()
    dxz_d = nc.declare_dram_parameter("dxz", [P, PAIRS * 3], BF16,
                                      isOutput=False)
    cj_d = nc.declare_dram_parameter("cj", [P, PAIRS * 64], BF16,
                                     isOutput=False)
    out_d = nc.declare_dram_parameter("out", [P, S * N_DESC * L_MAX], F32,
                                      isOutput=True)

    with tile.TileContext(nc) as tc:
        with tc.tile_pool(name="main", bufs=1) as pool:
            # activation biases must be const APs
            cvals = [math.pi / 2, 0.0]
            consts = pool.tile([P, len(cvals)], F32)
            for ci, cv in enumerate(cvals):
                nc.vector.memset(consts[:, ci:ci + 1], cv)
                nc.const_aps.aps[(F32, cv)] = consts[:, ci:ci + 1]

            ph1 = tc.tile_pool(name="ph1", bufs=1)
            cp = ph1.__enter__()

            dxyz = cp.tile([P, PAIRS * 3], BF16)   # [s, 3, j] rel. pos.
            nc.sync.dma_start(out=dxyz[:], in_=dxz_d[:])
            cj = cp.tile([P, PAIRS * 64], BF16)    # [k, s, d, j]
            # gate the big cj DMA behind dxyz via a WAW dependency: the
            # dummy copy reads dxyz, writes one element of cj, and the
            # DMA (which overwrites all of cj) must follow it.
            with nc.allow_low_precision(reason="dma gate dummy"):
                nc.vector.tensor_copy(out=cj[:, 0:1], in_=dxyz[:, 0:1])
            nc.scalar.dma_start(out=cj[:], in_=cj_d[:])

            # early memsets while waiting for posj
            Y = pool.tile([P, PAIRS * 16], BF16)   # [s, m, j]

            def y_slice(m, cnt=1):
                if cnt == 1:
                    return _ap(Y, m * M_NBR, [(16 * M_NBR, S), (1, M_NBR)])
                return _ap(Y, m * M_NBR,
                           [(16 * M_NBR, S), (M_NBR, cnt), (1, M_NBR)])

            Tall = cp.tile([P, K_MAX * PAIRS], BF16)   # [k][s,j]

            def t_slice(k):
                return _ap(Tall, k * PAIRS, [(1, PAIRS)])

            with nc.allow_low_precision(reason="bf16 basis"):
                nc.vector.memset(y_slice(0), 1.0)
                nc.vector.memset(t_slice(0), 1.0)

            # ---- geometry (f32): dxyz, r2, r, 1/r -----------------------
            C3 = PAIRS * 3
            SJ3 = 3 * M_NBR

            def c_slice(t, c):
                return _ap(t, c * M_NBR, [(SJ3, S), (1, M_NBR)])

            sq = cp.tile([P, C3], BF16)
            r2 = cp.tile([P, PAIRS], BF16)
            with nc.allow_low_precision(reason="bf16 geometry"):
                nc.vector.tensor_tensor(out=sq[:], in0=dxyz[:], in1=dxyz[:],
                                        op=ALU.mult)
                nc.vector.tensor_tensor(out=r2[:], in0=c_slice(sq, 0),
                                        in1=c_slice(sq, 1), op=ALU.add)
                nc.vector.tensor_tensor(
                    out=r2[:],
                    in0=_ap(r2, 0, [(M_NBR, S), (1, M_NBR)]),
                    in1=c_slice(sq, 2), op=ALU.add)
            # clamp r^2 at R_C^2: fc(R_C) = 0 exactly, so pairs beyond
            # the cutoff contribute 0 through fch regardless of u/tm1.
            with nc.allow_low_precision(reason="bf16 geometry"):
                nc.vector.tensor_scalar_min(out=r2[:], in0=r2[:],
                                            scalar1=R_C * R_C)
            rr = cp.tile([P, PAIRS], F32)
            nc.scalar.sqrt(out=rr[:], in_=r2[:])
            cosx = cp.tile([P, PAIRS], F32)
            nc.scalar.activation(out=cosx[:], in_=rr[:], func=AF.Sin,
                                 bias=math.pi / 2, scale=-math.pi / R_C)
            rinv = cp.tile([P, PAIRS], BF16)
            with nc.allow_low_precision(reason="bf16 reciprocal"):
                nc.vector.reciprocal(out=rinv[:], in_=rr[:])
            u = cp.tile([P, C3], BF16)             # [s, 3, j] unit vectors
            with nc.allow_low_precision(reason="bf16 unit vectors"):
                nc.vector.tensor_tensor(
                    out=u[:], in0=dxyz[:],
                    in1=_ap(rinv, 0, [(M_NBR, S), (0, 3), (1, M_NBR)]),
                    op=ALU.mult)

            # ---- Chebyshev recurrence in bf16 [k][s,j] ------------------
            tm1 = cp.tile([P, PAIRS], F32)
            nc.vector.tensor_scalar(out=tm1[:], in0=rr[:], scalar1=1.0 / R_C,
                                    scalar2=-1.0, op0=ALU.mult, op1=ALU.add)
            x2 = cp.tile([P, PAIRS], BF16)
            tsq = cp.tile([P, PAIRS], BF16)
            with nc.allow_low_precision(reason="bf16 chebyshev"):
                # T1 = 2*tm1^2 - 1; x2 = 2*T1 = 4*tm1^2 - 2 (from tsq,
                # independent of the T1 write for better ILP)
                nc.vector.tensor_tensor(out=tsq[:], in0=tm1[:], in1=tm1[:],
                                        op=ALU.mult)
                nc.vector.tensor_scalar(out=t_slice(1), in0=tsq[:],
                                        scalar1=2.0, scalar2=-1.0,
                                        op0=ALU.mult, op1=ALU.add)
                nc.vector.tensor_scalar(out=x2[:], in0=tsq[:],
                                        scalar1=4.0, scalar2=-2.0,
                                        op0=ALU.mult, op1=ALU.add)
                for k in range(2, K_MAX):
                    nc.vector.tensor_tensor(out=t_slice(k), in0=x2[:],
                                            in1=t_slice(k - 1), op=ALU.mult)
                    if k == 2:
                        nc.vector.tensor_scalar(out=t_slice(k),
                                                in0=t_slice(k), scalar1=-1.0,
                                                scalar2=None, op0=ALU.add)
                    else:
                        nc.vector.tensor_tensor(out=t_slice(k),
                                                in0=t_slice(k),
                                                in1=t_slice(k - 2),
                                                op=ALU.subtract)

            # fch = 0.25*cosx + 0.25  (= 0.5 * fc), bf16
            fch = cp.tile([P, PAIRS], BF16)
            with nc.allow_low_precision(reason="bf16 fch"):
                nc.vector.tensor_scalar(out=fch[:], in0=cosx[:], scalar1=0.25,
                                        scalar2=0.25, op0=ALU.mult,
                                        op1=ALU.add)

            # ---- radial basis f[k][s,j] = Tall[k] * fch -----------------
            # k-major like Tall, so this op is fully contiguous (2x mode).
            # (k=0 slice of Tall is 1, so f[0] = fch; the "+1" of the
            # reference basis is folded into c column 0 on the host)
            f = pool.tile([P, K_MAX * PAIRS], BF16)
            with nc.allow_low_precision(reason="bf16 radial basis"):
                nc.vector.tensor_tensor(
                    out=f[:], in0=Tall[:],
                    in1=_ap(fch, 0, [(0, K_MAX), (1, PAIRS)]),
                    op=ALU.mult)

            # ---- spherical harmonics Y[s,m,j] (bf16) --------------------
            # within-shell m order: l=2 -> [xy, xz, yz, z2, x2-y2]
            #                       l=3 -> [z3, x(5z2-1), y(5z2-1),
            #                               (x2-y2)z, xyz, tl5*x, tl6*y]
            def u_c(c, cnt=1):
                if cnt == 1:
                    return c_slice(u, c)
                return _ap(u, c * M_NBR, [(SJ3, S), (M_NBR, cnt), (1, M_NBR)])

            sc = cp.tile([P, PAIRS * 7], BF16)
            zs = cp.tile([P, PAIRS], BF16)

            def sc_t(i, cnt=1):
                if cnt == 1:
                    return _ap(sc, i * PAIRS, [(1, PAIRS)])
                return _ap(sc, i * PAIRS, [(M_NBR, S), (PAIRS, cnt),
                                           (1, M_NBR)])

            x2c, y2c, z2c, xyc, dxyc, tl = [sc_t(i) for i in range(6)]
            with nc.allow_low_precision(reason="bf16 Y"):
                nc.vector.tensor_copy(
                    out=y_slice(1, 3),
                    in_=_ap(u, 0, [(SJ3, S), (M_NBR, 3), (1, M_NBR)]))
                # x2c, y2c, z2c in one op
                nc.vector.tensor_tensor(out=sc_t(0, 3), in0=u_c(0, 3),
                                        in1=u_c(0, 3), op=ALU.mult)
                nc.vector.tensor_tensor(out=xyc, in0=u_c(0), in1=u_c(1),
                                        op=ALU.mult)
                nc.vector.tensor_scalar_mul(out=y_slice(4), in0=xyc,
                                            scalar1=SQ3)
                # Y5 = sqrt3*uz*ux, Y6 = sqrt3*uz*uy via pre-scaled uz
                # (STT is limited to 2 free dims; TT allows the pair)
                nc.vector.tensor_scalar_mul(out=zs[:], in0=u_c(2),
                                            scalar1=SQ3)
                nc.vector.tensor_tensor(
                    out=y_slice(5, 2),
                    in0=_ap(zs, 0, [(M_NBR, S), (0, 2), (1, M_NBR)]),
                    in1=u_c(0, 2), op=ALU.mult)
                nc.vector.tensor_scalar(out=y_slice(7), in0=z2c, scalar1=1.5,
                                        scalar2=-0.5, op0=ALU.mult,
                                        op1=ALU.add)
                nc.vector.tensor_tensor(out=dxyc, in0=x2c, in1=y2c,
                                        op=ALU.subtract)
                nc.vector.tensor_scalar_mul(out=y_slice(8), in0=dxyc,
                                            scalar1=SQ3 / 2)
                nc.vector.tensor_scalar(out=tl, in0=z2c, scalar1=2.5,
                                        scalar2=-1.5, op0=ALU.mult,
                                        op1=ALU.add)
                nc.vector.tensor_tensor(out=y_slice(9), in0=tl, in1=u_c(2),
                                        op=ALU.mult)
                nc.vector.tensor_scalar(out=tl, in0=z2c, scalar1=5.0 * C31,
                                        scalar2=-C31, op0=ALU.mult,
                                        op1=ALU.add)
                # Y10 = tl*ux, Y11 = tl*uy in one op
                nc.vector.tensor_tensor(
                    out=y_slice(10, 2),
                    in0=_ap(sc, 5 * PAIRS, [(M_NBR, S), (0, 2), (1, M_NBR)]),
                    in1=u_c(0, 2), op=ALU.mult)
                # Y12 = (C32/2*dxyc)*uz, Y13 = (C32*xyc)*uz: prescale
                # into adjacent scratch then one paired TT
                nc.vector.tensor_scalar_mul(out=sc_t(4), in0=dxyc,
                                            scalar1=C32 / 2)
                nc.vector.tensor_scalar_mul(out=sc_t(6), in0=xyc,
                                            scalar1=C32)
                nc.vector.tensor_tensor(
                    out=y_slice(12, 2),
                    in0=_ap(sc, 4 * PAIRS, [(M_NBR, S), (2 * PAIRS, 2),
                                            (1, M_NBR)]),
                    in1=_ap(u, 2 * M_NBR, [(SJ3, S), (0, 2), (1, M_NBR)]),
                    op=ALU.mult)
                # Y14 = (-C33*(3y2-x2))*ux, Y15 = (C33*(3x2-y2))*uy via
                # prescales into slots 4/6 (dead after Y12/13) + pair TT
                nc.vector.scalar_tensor_tensor(out=tl, in0=y2c, scalar=3.0,
                                               in1=x2c, op0=ALU.mult,
                                               op1=ALU.subtract)
                nc.vector.tensor_scalar_mul(out=sc_t(4), in0=tl,
                                            scalar1=-C33)
                nc.vector.scalar_tensor_tensor(out=tl, in0=x2c, scalar=3.0,
                                               in1=y2c, op0=ALU.mult,
                                               op1=ALU.subtract)
                nc.vector.tensor_scalar_mul(out=sc_t(6), in0=tl,
                                            scalar1=C33)
                nc.vector.tensor_tensor(
                    out=y_slice(14, 2),
                    in0=_ap(sc, 4 * PAIRS, [(M_NBR, S), (2 * PAIRS, 2),
                                            (1, M_NBR)]),
                    in1=u_c(0, 2), op=ALU.mult)

            # ---- g[s,d,j] = sum_k cj[k,s,d,j] * f[k,s,j] ----------------
            # cj is host-marshaled k-major so the whole k tree is flat
            # contiguous adds (2x mode) and g lands in [s,d,j] directly.
            x2t = cp.tile([P, PAIRS * 64], BF16)     # [k, s, d, j]
            a4 = cp.tile([P, PAIRS * 32], BF16)      # [k4, s, d, j]
            a2 = cp.tile([P, PAIRS * 16], BF16)      # [k2, s, d, j]
            g = pool.tile([P, PAIRS * N_DESC], BF16)   # [s, d, j]
            with nc.allow_low_precision(reason="bf16 contraction"):
                nc.vector.tensor_tensor(
                    out=_ap(x2t, 0, [(1600, K_MAX), (160, S), (20, N_DESC),
                                     (1, M_NBR)]),
                    in0=_ap(cj, 0, [(1600, K_MAX), (160, S), (20, N_DESC),
                                    (1, M_NBR)]),
                    in1=_ap(f, 0, [(PAIRS, K_MAX), (M_NBR, S), (0, N_DESC),
                                   (1, M_NBR)]),
                    op=ALU.mult)
                nc.vector.tensor_tensor(
                    out=a4[:], in0=x2t[:, 0:6400], in1=x2t[:, 6400:12800],
                    op=ALU.add)
                nc.vector.tensor_tensor(
                    out=a2[:], in0=a4[:, 0:3200], in1=a4[:, 3200:6400],
                    op=ALU.add)
                nc.vector.tensor_tensor(
                    out=g[:], in0=a2[:, 0:1600], in1=a2[:, 1600:3200],
                    op=ALU.add)

            ph1.__exit__(None, None, None)
            ph2 = tc.tile_pool(name="ph2", bufs=1)
            p2 = ph2.__enter__()

            # ---- A[s,d,m] = sum_j g * Y  (tree over j = 8+8+4) ----------
            # m has 17 slots: 16 spherical harmonics plus g^2 (so the
            # tree also produces B[s,d] = A[s,d,16] for free).
            MM = 17
            xa = p2.tile([P, S * N_DESC * MM * M_NBR], BF16)  # [s,d,m,j]
            t8 = p2.tile([P, S * N_DESC * MM * 8], BF16)
            t4 = p2.tile([P, S * N_DESC * MM * 4], BF16)
            t4b = p2.tile([P, S * N_DESC * MM * 4], BF16)
            t2 = p2.tile([P, S * N_DESC * MM * 2], BF16)
            A = pool.tile([P, S * N_DESC * MM], F32)          # [s, d, m17]
            SX, DX = N_DESC * MM * M_NBR, MM * M_NBR
            with nc.allow_low_precision(reason="bf16 outer product"):
                # g^2 into m=16 (contiguous both sides -> 2x mode)
                nc.vector.tensor_tensor(
                    out=_ap(xa, 16 * M_NBR, [(SX, S), (DX, N_DESC),
                                             (1, M_NBR)]),
                    in0=_ap(g, 0, [(160, S), (20, N_DESC), (1, M_NBR)]),
                    in1=_ap(g, 0, [(160, S), (20, N_DESC), (1, M_NBR)]),
                    op=ALU.mult)
                nc.vector.tensor_tensor(
                    out=_ap(xa, 0, [(SX, S), (DX, N_DESC), (20, 16),
                                    (1, 20)]),
                    in0=_ap(g, 0, [(160, S), (20, N_DESC), (0, 16), (1, 20)]),
                    in1=_ap(Y, 0, [(320, S), (0, N_DESC), (20, 16), (1, 20)]),
                    op=ALU.mult)
                nc.vector.tensor_tensor(
                    out=_ap(t8, 0, [(MM * 64, S), (MM * 8, N_DESC), (8, MM),
                                    (1, 8)]),
                    in0=_ap(xa, 0, [(SX, S), (DX, N_DESC), (20, MM),
                                    (1, 8)]),
                    in1=_ap(xa, 8, [(SX, S), (DX, N_DESC), (20, MM),
                                    (1, 8)]),
                    op=ALU.add)
                nc.vector.tensor_tensor(
                    out=_ap(t4, 0, [(MM * 32, S), (MM * 4, N_DESC), (4, MM),
                                    (1, 4)]),
                    in0=_ap(t8, 0, [(MM * 64, S), (MM * 8, N_DESC), (8, MM),
                                    (1, 4)]),
                    in1=_ap(t8, 4, [(MM * 64, S), (MM * 8, N_DESC), (8, MM),
                                    (1, 4)]),
                    op=ALU.add)
                nc.vector.tensor_tensor(
                    out=_ap(t4b, 0, [(MM * 32, S), (MM * 4, N_DESC), (4, MM),
                                     (1, 4)]),
                    in0=_ap(t4, 0, [(MM * 32, S), (MM * 4, N_DESC), (4, MM),
                                    (1, 4)]),
                    in1=_ap(xa, 16, [(SX, S), (DX, N_DESC), (20, MM),
                                     (1, 4)]),
                    op=ALU.add)
                nc.vector.tensor_tensor(
                    out=_ap(t2, 0, [(MM * 16, S), (MM * 2, N_DESC), (2, MM),
                                    (1, 2)]),
                    in0=_ap(t4b, 0, [(MM * 32, S), (MM * 4, N_DESC), (4, MM),
                                     (1, 2)]),
                    in1=_ap(t4b, 2, [(MM * 32, S), (MM * 4, N_DESC), (4, MM),
                                     (1, 2)]),
                    op=ALU.add)
            nc.vector.tensor_tensor(
                out=_ap(A, 0, [(N_DESC * MM, S), (MM, N_DESC), (1, MM)]),
                in0=_ap(t2, 0, [(MM * 16, S), (MM * 2, N_DESC), (2, MM)]),
                in1=_ap(t2, 1, [(MM * 16, S), (MM * 2, N_DESC), (2, MM)]),
                op=ALU.add)

            if debug:
                for nm, t, dt in [("f", f, BF16), ("Y", Y, BF16),
                                  ("g", g, BF16), ("A", A, F32)]:
                    dd = nc.declare_dram_parameter(
                        "d_" + nm, [P, t.shape[1]], dt, isOutput=True)
                    nc.sync.dma_start(out=dd[:], in_=t[:])

            # ---- q[s,d,l] = sum_{m in shell l} A^2 - B ------------------
            # two atom-halves so the first half's output DMA overlaps the
            # second half's compute
            Asq = pool.tile([P, S * N_DESC * 16], F32)
            outq = pool.tile([P, S * N_DESC * L_MAX], F32)
            q2la = pool.tile([P, S * N_DESC * L_MAX], F32)
            SH_ = S // 2
            for h in range(2):
                sqo, ao, qo = h * SH_ * 128, h * SH_ * N_DESC * MM,                     h * SH_ * N_DESC * L_MAX
                nc.vector.tensor_tensor(
                    out=_ap(Asq, sqo, [(128, SH_), (16, N_DESC), (1, 16)]),
                    in0=_ap(A, ao, [(N_DESC * MM, SH_), (MM, N_DESC),
                                    (1, 16)]),
                    in1=_ap(A, ao, [(N_DESC * MM, SH_), (MM, N_DESC),
                                    (1, 16)]),
                    op=ALU.mult)
                for l in range(L_MAX):
                    cnt = SHELL_OFF[l + 1] - SHELL_OFF[l]
                    nc.vector.tensor_reduce(
                        out=_ap(q2la, qo + l, [(N_DESC * L_MAX, SH_),
                                               (L_MAX, N_DESC)]),
                        in_=_ap(Asq, sqo + SHELL_OFF[l],
                                [(N_DESC * 16, SH_), (16, N_DESC), (1, cnt)]),
                        axis=AX.X, op=ALU.add)
                nc.vector.tensor_tensor(
                    out=_ap(outq, qo, [(1, SH_ * N_DESC * L_MAX)]),
                    in0=_ap(q2la, qo, [(1, SH_ * N_DESC * L_MAX)]),
                    in1=_ap(A, ao + 16, [(N_DESC * MM, SH_), (MM, N_DESC),
                                         (0, L_MAX)]),
                    op=ALU.subtract)
                nq = SH_ * N_DESC * L_MAX
                nc.sync.dma_start(out=out_d[:, qo:qo + nq],
                                  in_=outq[:, qo:qo + nq])
            ph2.__exit__(None, None, None)
    nc.finalize()
    return nc


def make_inputs(types, positions, angular_neighbors, c_table):
    types = np.asarray(types).astype(np.int64)
    positions = np.ascontiguousarray(np.asarray(positions, dtype=np.float32))
    nbr = np.asarray(angular_neighbors).astype(np.int64)
    c_table = np.asarray(c_table, dtype=np.float32)
    import ml_dtypes

    pad = NTOT - N_ATOMS
    types_pad = np.concatenate([types, np.repeat(types[-1:], pad, 0)], 0)
    pos_pad = np.concatenate([positions, np.repeat(positions[-1:], pad, 0)],
                             0)
    nbr_pad = np.concatenate([nbr, np.repeat(nbr[-1:], pad, 0)], 0)

    # per-(t_i,t_j) c rows in [d, k] order, sqrt(0.5)-scaled.
    # Column 0 absorbs sum_k c[d,k] (device basis is [fch, T_1*fch, ...]).
    c_adj = c_table.astype(np.float64).copy()
    c_adj[..., 0] += c_table.astype(np.float64).sum(-1)
    c16 = (c_adj * math.sqrt(0.5)).astype(ml_dtypes.bfloat16)  # [4,4,8,8]

    pvec = np.arange(P)
    svec = np.arange(S)
    in_maps = []
    for c in range(NCORES):
        atom = c * CA + pvec[:, None] * S + svec[None, :]       # [P, S]
        nbrs = nbr_pad[atom]                                    # [P, S, 20]
        # relative positions, component-major [P, S, 3, M], bf16
        dxz = (pos_pad[nbrs] - pos_pad[atom][:, :, None, :]).astype(
            np.float32)
        dxz = dxz.transpose(0, 1, 3, 2).reshape(P, PAIRS * 3)
        dxz = dxz.astype(ml_dtypes.bfloat16)
        tj = types_pad[nbrs]                                    # [P, S, 20]
        # [P,S,M,d,k] -> k-major [P,k,S,d,M(j)]
        cjf = c16[types_pad[atom][:, :, None], tj]
        cj = cjf.transpose(0, 4, 1, 3, 2).reshape(P, PAIRS * 64)
        in_maps.append({
            "dxz": np.ascontiguousarray(dxz),
            "cj": np.ascontiguousarray(cj),
        })
    return in_maps


_NC_CACHE = None


def kernel(types, positions, angular_neighbors, c_table):
    global _NC_CACHE
    in_maps = make_inputs(types, positions, angular_neighbors, c_table)
    if _NC_CACHE is None:
        _NC_CACHE = build_nc()
    res = run_bass_kernel_spmd(_NC_CACHE, in_maps,
                               core_ids=list(range(NCORES)))
    outs = [res.results[c]["out"].reshape(CA, N_DESC, L_MAX)
            for c in range(NCORES)]
    q = np.concatenate(outs, 0)[:N_ATOMS]
    return np.ascontiguousarray(q.astype(np.float32))


if __name__ == "__main__":
    z = np.load("/tmp/ref_cache.npz")
    inputs = {k: z[k] for k in
              ("types", "positions", "angular_neighbors", "c_table")}
    exp = z["exp"]
    act = kernel(**inputs)
    rel = np.linalg.norm(act - exp) / np.linalg.norm(exp)
    print("Relative error:", rel)
